# revision 34
# baseline (speedup 1.0000x reference)
"""EquivariantCrossAttention Trainium2 kernel (8 NeuronCores, SPMD).

kernel(**inputs) takes the FULL unsharded inputs from reference's
setup_inputs() and returns the FULL (B, N, DH) float32 output.

Sharding: flattened query axis (B*N = 4096) split into 8 shards of 512
queries; core i gets queries [512*i, 512*(i+1)) plus its batch's latent
tables. Weights replicated.

Hardcoded problem shapes: B=2 N=2048 L=1024 K=16 CD=2 H=4 DH=128 HD=512.

Algebraic folds done host-side (exact):
  - LayerNorm affines folded into the following Dense weights
  - attention SCALE and eq_w2 folded into wq (W_qm = eq_w2 @ (wq*SCALE))
  - q.k per head via M_h = W_qm_h @ wk_h^T:
      att = g2^T M_h cg + g2.w2v_h + cg.w1v_h + const_h
    The cg.w1v_h term is precomputed per latent on host and gathered.
  - mFFN dense2 and wo merged (W_mo_h = mw2' @ wo_h) and moved after the
    attention sum (softmax weights sum to 1; dense2 affine)
  - mFFN LN normalization folded into attention weights:
      sum_k att*LN(g) = sum_k (att*rstd) g - sum_k att*mean*rstd
  - RFF: t = x@Bs - p@Bs; p@Bs computed on device from hi/lo bf16 split
    of p (3-term product vs hi/lo split of Bs rows);
    sin(2pi t) = Sin(2pi(t - rint t)); cos via +0.25 shift.
  - per-latent gather table row (256 bf16 = 512B):
      [ c (128) | w1vc_h (4) | p0hi p0lo p0hi p1hi p1lo p1hi |
        1/sigma^2 | pad ]
    gathered once per query tile (2048 idxs) with transpose=True.
  - rstd = exp(-0.5*ln(var+eps)) so LN rstds share the ln/exp ACT table
    set with the softmax Exp; mrstd/att_e of chunk c are emitted in
    chunk c+1's ln/exp block (software pipelining) so the scalar engine
    switches table sets only ~2.5x per chunk.

Device structure per core: 4 query tiles x 128 queries; per tile:
scores via PE (|p|^2 folded in via a ones-row) -> top-16 via DVE
max/max_index/match_replace -> one transposed dma_gather -> hoisted
RFF/sin features for all 4 chunks -> 4 chunks of 512 rows (32 queries x
16 neighbors, q-major) through the fused MLP/attention pipeline.
"""

import sys

sys.path.insert(0, "/opt/trn_rl_repo")

import numpy as np
import ml_dtypes

import concourse.bass as bass
import concourse.bacc as bacc
import concourse.mybir as mybir
import concourse.tile as tile
from concourse.masks import make_identity

F32 = mybir.dt.float32
BF16 = mybir.dt.bfloat16
U32 = mybir.dt.uint32
I32 = mybir.dt.int32
I16 = mybir.dt.int16
AF = mybir.ActivationFunctionType
OP = mybir.AluOpType
AX = mybir.AxisListType

B, N, L, K, CD, H, DH, D = 2, 2048, 1024, 16, 2, 4, 128, 128
HD = H * DH
FQ = 2.0
FV = 2.0
SCALE = 1.0 / float(np.sqrt(DH))
NCORES = 8
NQ = (B * N) // NCORES  # queries per core = 512
QT = NQ // 128  # query tiles per core = 4
NCH = 4  # chunks per query tile
CQ = 128 // NCH  # queries per chunk = 32
CR = CQ * K  # rows per chunk = 512
TR = 128 * K  # rows per tile = 2048
GELU = AF.Gelu_apprx_tanh
TWO_PI = 2.0 * np.pi
EPS = 1e-6

WSPECS = [
    ("rff", [CD, 128], F32),
    ("rff6", [38, 128], BF16),
    ("sel4", [4, 128], BF16),
    ("eq_w1", [128, 128], BF16),
    ("eq_b1", [128, 1], F32),
    ("Mq", [128, 512], BF16),
    ("w2v", [128, 4], BF16),
    ("attconst", [128, 1], F32),
    ("ev_w1", [128, 128], BF16),
    ("ev_b1", [128, 1], F32),
    ("ev_w2", [128, 128], BF16),
    ("ev_b2", [128, 1], F32),
    ("ivw1", [128, 128], BF16),
    ("ivb1", [128, 1], F32),
    ("ivw2g", [128, 512], BF16),
    ("wv", [128, 512], BF16),
    ("WA", [128, 512], BF16),
    ("WB", [128, 512], BF16),
    ("mw1", [128, 128], BF16),
    ("mb1p", [128, 4], F32),
    ("Wmo", [128, 512], BF16),
    ("bmo", [128, 1], F32),
    ("cw1", [128, 128], BF16),
    ("cb1", [128, 1], F32),
    ("cw2g", [128, 128], BF16),
    ("cw2b", [128, 128], BF16),
    ("cb2g1", [128, 1], F32),
    ("cb2b", [128, 1], F32),
]


def _bcast_inner(ap, n):
    """[.., Q] AP -> [.., Q, n] with a stride-0 inner dim (free broadcast)."""
    newap = [list(p) for p in ap.ap] + [[0, n]]
    return bass.AP(ap.tensor, ap.offset, newap)


def build_program():
    nc = bacc.Bacc()

    x_d = nc.declare_dram_parameter("x", [NQ, 3], F32, isOutput=False)
    xh_d = nc.declare_dram_parameter("xh", [NQ, DH], F32, isOutput=False)
    ctbl_d = nc.declare_dram_parameter("ctbl", [L, 2 * D], BF16, isOutput=False)
    p2t_d = nc.declare_dram_parameter("p2t", [3, L], F32, isOutput=False)
    w_d = {}
    for name, shape, dt in WSPECS:
        w_d[name] = nc.declare_dram_parameter(name, shape, dt, isOutput=False)
    out_d = nc.declare_dram_parameter("out", [NQ, DH], F32, isOutput=True)

    with tile.TileContext(nc) as tc:
        _emit(nc, tc, x_d, xh_d, ctbl_d, p2t_d, w_d, out_d)
    nc.compile()
    return nc


def _emit(nc, tc, x_d, xh_d, ctbl_d, p2t_d, w_d, out_d):
    const = tc.alloc_tile_pool(name="const", bufs=1)
    wpool = tc.alloc_tile_pool(name="wpool", bufs=1)
    core = tc.alloc_tile_pool(name="core", bufs=1)
    tl = tc.alloc_tile_pool(name="tl", bufs=1)
    ck = tc.alloc_tile_pool(name="ck", bufs=1)
    psp = tc.alloc_tile_pool(name="psp", bufs=1, space="PSUM")
    _pools = [const, wpool, core, tl, ck, psp]

    _psn = [0]

    def PS(shape, tag, bufs, dtype=F32):
        _psn[0] += 1
        return psp.tile(
            shape, dtype, space="PSUM", tag=tag, bufs=bufs, name=f"ps_{tag}_{_psn[0]}"
        )

    # ---------- constants ----------
    ident = const.tile([128, 128], F32)
    make_identity(nc, ident[:])

    ones_col_bf = const.tile([128, 1], BF16)
    nc.vector.memset(ones_col_bf[:], 1.0)
    inv128_bf = const.tile([128, 1], BF16)
    nc.vector.memset(inv128_bf[:], 1.0 / 128.0)
    ones_row_bf = const.tile([1, 128], BF16)
    nc.vector.memset(ones_row_bf[:], 1.0)
    onesmat_bf = const.tile([128, 128], BF16)
    nc.vector.memset(onesmat_bf[:], 1.0)
    eps_col = const.tile([128, 1], F32)
    nc.vector.memset(eps_col[:], EPS)
    zeros_row_bf = const.tile([1, 128], BF16)
    nc.vector.memset(zeros_row_bf[:], 0.0)
    zeros_row512_bf = const.tile([1, 512], BF16)
    nc.vector.memset(zeros_row512_bf[:], 0.0)

    # ---------- weights ----------
    W = {}
    for name, shape, dt in WSPECS:
        wt = wpool.tile(shape, dt, name=f"w_{name}", tag=f"w_{name}")
        nc.sync.dma_start(out=wt[:], in_=w_d[name][:])
        W[name] = wt

    def Wh(name, h, w=128):
        return W[name][:, h * w : (h + 1) * w]

    # ---------- per-core precompute ----------
    # x arrives with a ones column appended (for the |p|^2 fold in scores)
    x_sb = core.tile([128, QT, 3], F32)
    nc.sync.dma_start(out=x_sb[:], in_=x_d[:].rearrange("(t q) c -> q t c", q=128))
    xsq = core.tile([128, QT], F32)
    xs2 = core.tile([128, QT, CD], F32)
    nc.vector.tensor_tensor(
        out=xs2[:], in0=x_sb[:, :, 0:CD], in1=x_sb[:, :, 0:CD], op=OP.mult
    )
    nc.vector.tensor_reduce(out=xsq[:], in_=xs2[:], axis=AX.X, op=OP.add)

    x_fm = core.tile([3, NQ], F32)
    for t in range(QT):
        tp = PS([3, 128], "tr", 1)
        nc.tensor.transpose(out=tp[:], in_=x_sb[:, t, :], identity=ident[:])
        nc.vector.tensor_copy(out=x_fm[:, 128 * t : 128 * (t + 1)], in_=tp[:])

    p2_fm = core.tile([3, L], F32)
    nc.sync.dma_start(out=p2_fm[:], in_=p2t_d[:])

    # ---------- cFFN on x_h (512 queries at once) ----------
    xh_fm = core.tile([128, NQ], BF16)
    xh_rm = core.tile([128, QT, DH], F32)
    nc.sync.dma_start(out=xh_rm[:], in_=xh_d[:].rearrange("(t q) c -> q t c", q=128))
    for t in range(QT):
        tp = PS([128, 128], "tr", 1)
        nc.tensor.transpose(out=tp[:], in_=xh_rm[:, t, :], identity=ident[:])
        nc.vector.tensor_copy(out=xh_fm[:, 128 * t : 128 * (t + 1)], in_=tp[:])

    c1ps = PS([128, NQ], "mm", 3)
    nc.tensor.matmul(out=c1ps[:], lhsT=W["cw1"][:], rhs=xh_fm[:], start=True, stop=True)
    c1 = core.tile([128, NQ], BF16)
    nc.scalar.activation(out=c1[:], in_=c1ps[:], func=GELU, bias=W["cb1"][:])

    cmean = PS([128, NQ], "rows4", 2)
    nc.tensor.matmul(out=cmean[0:1, :], lhsT=inv128_bf[:], rhs=c1[:], start=True, stop=True)
    c1sq = core.tile([128, NQ], BF16)
    nc.vector.tensor_tensor(out=c1sq[:], in0=c1[:], in1=c1[:], op=OP.mult)
    cmsq = PS([128, NQ], "rows4", 2)
    nc.tensor.matmul(out=cmsq[0:1, :], lhsT=inv128_bf[:], rhs=c1sq[:], start=True, stop=True)

    cm2 = core.tile([1, NQ], F32)
    nc.scalar.square(out=cm2[:], in_=cmean[0:1, :])
    cvar = core.tile([1, NQ], F32)
    nc.vector.tensor_tensor(out=cvar[:], in0=cmsq[0:1, :], in1=cm2[:], op=OP.subtract)
    # rstd = exp(-0.5*ln(var+eps)); stays in the ln/exp table set
    clnv = core.tile([1, NQ], F32)
    nc.scalar.activation(out=clnv[:], in_=cvar[:], func=AF.Ln, bias=eps_col[0:1, :])
    crstd_bf = core.tile([1, NQ], BF16)
    nc.scalar.activation(out=crstd_bf[:], in_=clnv[:], func=AF.Exp, scale=-0.5)
    cmr_bf = core.tile([1, NQ], BF16)
    nc.vector.tensor_tensor(out=cmr_bf[:], in0=cmean[0:1, :], in1=crstd_bf[:], op=OP.mult)
    crbc = PS([128, NQ], "mm", 3)
    nc.tensor.matmul(out=crbc[:], lhsT=ones_row_bf[:], rhs=crstd_bf[:], start=True, stop=True)
    cmbc = PS([128, NQ], "mm", 3)
    nc.tensor.matmul(out=cmbc[:], lhsT=ones_row_bf[:], rhs=cmr_bf[:], start=True, stop=True)
    z1c = core.tile([128, NQ], BF16)
    nc.vector.tensor_tensor(out=z1c[:], in0=c1[:], in1=crbc[:], op=OP.mult)
    zc = core.tile([128, NQ], BF16)
    nc.vector.tensor_tensor(out=zc[:], in0=z1c[:], in1=cmbc[:], op=OP.subtract)

    gp1 = core.tile([128, NQ], BF16)
    modadd = core.tile([128, NQ], BF16)
    gps_ = PS([128, NQ], "mm", 3)
    nc.tensor.matmul(out=gps_[:], lhsT=W["cw2g"][:], rhs=zc[:], start=True, stop=True)
    nc.scalar.activation(out=gp1[:], in_=gps_[:], func=AF.Identity, bias=W["cb2g1"][:])
    btps = PS([128, NQ], "mm", 3)
    nc.tensor.matmul(out=btps[:], lhsT=W["cw2b"][:], rhs=zc[:], start=True, stop=True)
    bt = core.tile([128, NQ], BF16)
    nc.scalar.activation(out=bt[:], in_=btps[:], func=AF.Identity, bias=W["cb2b"][:])
    ma1 = core.tile([128, NQ], BF16)
    nc.vector.tensor_scalar(
        out=ma1[:], in0=gp1[:], scalar1=W["ev_b2"][:], scalar2=None, op0=OP.mult
    )
    nc.vector.tensor_tensor(out=modadd[:], in0=ma1[:], in1=bt[:], op=OP.add)

    # deferred-stage state carried across chunks: (attps, mmean, msqp, gms,
    # qsl, zacc, den_t); tile epilogues are likewise deferred until after
    # the tile's chunk-3 deferred stage has been emitted.
    carry = [None]
    pending_epi = [None]

    def deferred_stage(state):
        """ln/exp block + weighted-sum DVE work for a completed chunk.

        Emitted one chunk later so mrstd/att_e share the ln/exp ACT table
        set with the next chunk's irstd."""
        attps, mmean, msqp, gms, qsl, zacc, den_t = state

        # mm2/mvar/mrstd: only rows {0,32,64,96} carry real stats; other
        # rows hold stale/garbage psum values whose results are never read.
        mm2 = ck.tile([128, CR], F32, tag="mm2")
        nc.scalar.square(out=mm2[:], in_=mmean[:])
        mvar = ck.tile([128, CR], F32, tag="mvar")
        nc.vector.tensor_tensor(out=mvar[:], in0=msqp[:], in1=mm2[:], op=OP.subtract)
        mlnv = ck.tile([128, CR], F32, tag="mlnv")
        nc.scalar.activation(out=mlnv[:], in_=mvar[:], func=AF.Ln, bias=eps_col[:])
        mrstd = ck.tile([128, CR], BF16, tag="mrstd")
        nc.scalar.activation(out=mrstd[:], in_=mlnv[:], func=AF.Exp, scale=-0.5)

        att_e = ck.tile([128, CR], BF16, tag="att_e")
        nc.scalar.activation(out=att_e[:], in_=attps[:], func=AF.Exp, bias=W["attconst"][:])
        nc.vector.tensor_reduce(
            out=den_t[:, qsl], in_=att_e[:].rearrange("p (a b) -> p a b", a=CQ),
            axis=AX.X, op=OP.add,
        )

        a2 = ck.tile([128, CR], BF16, tag="a2")
        nc.vector.tensor_tensor(out=a2[:], in0=att_e[:], in1=mrstd[:], op=OP.mult)
        # a3 = att_e * mean * rstd = a2 * mean  (psum operand)
        a3 = ck.tile([128, CR], BF16, tag="a3")
        nc.vector.tensor_tensor(out=a3[:], in0=a2[:], in1=mmean[:], op=OP.mult)
        s3bf = ck.tile([128, CQ], BF16, tag="s3bf")
        with nc.allow_low_precision(reason="16-term bf16 row sum; folded into bf16 matmul anyway"):
            nc.vector.tensor_reduce(
                out=s3bf[:], in_=a3[:].rearrange("p (a b) -> p a b", a=CQ), axis=AX.X,
                op=OP.add,
            )

        for h in range(H):
            a2bc = PS([128, CR], "mm", 3)
            nc.tensor.matmul(
                out=a2bc[:], lhsT=onesmat_bf[32 * h : 32 * h + 1, :],
                rhs=a2[32 * h : 32 * h + 1, :], start=True, stop=True,
                tile_position=(32 * h, 0),
            )
            zp = ck.tile([128, CR], BF16, tag="zp", bufs=2)
            nc.vector.tensor_tensor(out=zp[:], in0=gms[h][:], in1=a2bc[:], op=OP.mult)
            nc.vector.tensor_reduce(
                out=zacc[h][:, qsl], in_=zp[:].rearrange("p (a b) -> p a b", a=CQ),
                axis=AX.X, op=OP.add,
            )
            s3bc = PS([128, CQ], "tr", 1)
            nc.tensor.matmul(
                out=s3bc[:], lhsT=onesmat_bf[32 * h : 32 * h + 1, :],
                rhs=s3bf[32 * h : 32 * h + 1, :], start=True, stop=True,
                tile_position=(32 * h, 0),
            )
            nc.vector.tensor_tensor(
                out=zacc[h][:, qsl], in0=zacc[h][:, qsl], in1=s3bc[:], op=OP.subtract
            )

    def emit_epilogue(state):
        """Softmax denominator fold + output matmul/transpose/store for a
        finished tile (emitted after the tile's chunk-3 deferred stage)."""
        zacc, den_t, qs = state
        rden_t = tl.tile([128, 128], F32, tag="rden_t", bufs=2)
        nc.vector.reciprocal(out=rden_t[:], in_=den_t[:])
        rdbf = tl.tile([128, 128], BF16, tag="rdbf", bufs=2)
        nc.vector.tensor_copy(out=rdbf[:], in_=rden_t[:])
        for h in range(H):
            rdbc = PS([128, 128], "tr", 1)
            nc.tensor.matmul(
                out=rdbc[:], lhsT=onesmat_bf[32 * h : 32 * h + 1, :],
                rhs=rdbf[32 * h : 32 * h + 1, :], start=True, stop=True,
                tile_position=(32 * h, 0),
            )
            nc.vector.tensor_tensor(out=zacc[h][:], in0=zacc[h][:], in1=rdbc[:], op=OP.mult)

        outps = PS([128, 128], "tr", 1)
        for h in range(H):
            zbf = tl.tile([128, 128], BF16, tag="zbf")
            nc.vector.tensor_copy(out=zbf[:], in_=zacc[h][:])
            nc.tensor.matmul(
                out=outps[:], lhsT=Wh("Wmo", h), rhs=zbf[:], start=(h == 0), stop=(h == H - 1)
            )
        outsb = tl.tile([128, 128], F32, tag="outsb")
        nc.scalar.activation(out=outsb[:], in_=outps[:], func=AF.Identity, bias=W["bmo"][:])
        trp = PS([128, 128], "tr", 1)
        nc.tensor.transpose(out=trp[:], in_=outsb[:], identity=ident[:])
        outrm = tl.tile([128, 128], F32, tag="outrm")
        nc.vector.tensor_copy(out=outrm[:], in_=trp[:])
        nc.sync.dma_start(out=out_d[qs, :], in_=outrm[:])

    # ---------- per query tile ----------
    for t in range(QT):
        qs = slice(128 * t, 128 * (t + 1))

        # scores (two 512-wide halves; |p|^2 folded via ones row) + top-16
        scores = tl.tile([128, L], F32, tag="scores")
        for s in range(2):
            sl = slice(512 * s, 512 * (s + 1))
            scps = PS([128, 512], "mm", 3)
            nc.tensor.matmul(out=scps[:], lhsT=x_fm[:, qs], rhs=p2_fm[:, sl], start=True, stop=True)
            nc.scalar.copy(out=scores[:, sl], in_=scps[:])
        vals = tl.tile([128, K], F32, tag="vals", bufs=2)
        idxs = tl.tile([128, K], U32, tag="idxs", bufs=2)
        scr2 = tl.tile([128, L], F32, tag="scr2")
        nc.vector.max(out=vals[:, 0:8], in_=scores[:])
        nc.vector.max_index(out=idxs[:, 0:8], in_max=vals[:, 0:8], in_values=scores[:])
        nc.vector.match_replace(
            out=scr2[:], in_to_replace=vals[:, 0:8], in_values=scores[:], imm_value=-1e30
        )
        nc.vector.max(out=vals[:, 8:16], in_=scr2[:])
        nc.vector.max_index(out=idxs[:, 8:16], in_max=vals[:, 8:16], in_values=scr2[:])

        # index prep for dma_gather: idx16[k, q] (int16) replicated across
        # the 8 gpsimd cores' 16-partition blocks
        idxf = tl.tile([128, K], F32, tag="idxf", bufs=2)
        nc.vector.tensor_copy(out=idxf[:], in_=idxs[:])
        idxt_ps = PS([K, 128], "tr", 1)
        nc.tensor.transpose(out=idxt_ps[:], in_=idxf[:], identity=ident[:])
        idx16 = tl.tile([128, 128], I16, tag="idx16", bufs=2)
        nc.vector.tensor_copy(out=idx16[0:16, :], in_=idxt_ps[:])
        for b in range(1, 8):
            nc.sync.dma_start(out=idx16[16 * b : 16 * b + 16, :], in_=idx16[0:16, :])

        # ONE transposed gather for the whole tile (2048 idxs, 512B rows):
        # cfm [128, 2, 2048]: block 0 = c features; block 1 rows 0-3 =
        # w1v.c per head, rows 32-37 = p hi/lo splits, row 64 = 0.5/sigma^2
        # (row choices keep every consumer's partition bases legal/aligned)
        cfm_cs = []
        for gc in range(NCH):
            cfm_c = tl.tile([128, 2, CR], BF16, tag=f"cfm{gc}", bufs=2,
                            name=f"cfm{gc}_{t}")
            nc.gpsimd.dma_gather(
                out_ap=cfm_c[:], in_ap=ctbl_d[:],
                idxs_ap=idx16[:, 32 * gc : 32 * gc + 32], num_idxs=CR,
                num_idxs_reg=CR, elem_size=2 * D, transpose=True,
            )
            cfm_cs.append(cfm_c)

        # -d^2 row (q-major), DMA-flattened INTO partition 64 so the gw
        # multiply is partition-base-aligned with the gathered 0.5/sigma^2
        negd2 = tl.tile([128, K], BF16, tag="negd2", bufs=2)
        nc.vector.tensor_scalar(
            out=negd2[:], in0=vals[:], scalar1=xsq[:, t : t + 1], scalar2=None,
            op0=OP.subtract,
        )
        negd2_row = tl.tile([65, TR], BF16, tag="negd2_row", bufs=2)
        nc.sync.dma_start(out=negd2_row[64:65, :], in_=negd2[:])

        # gaussian window row for the whole tile: gw = -d^2 * 0.5/sigma^2
        gw_t = tl.tile([65, TR], BF16, tag="gw_t", bufs=2)
        for gc in range(NCH):
            nc.vector.tensor_tensor(
                out=gw_t[64:65, CR * gc : CR * (gc + 1)],
                in0=negd2_row[64:65, CR * gc : CR * (gc + 1)],
                in1=cfm_cs[gc][64:65, 1, :], op=OP.mult,
            )

        # t_x = x @ Bs for this tile [128, 128]
        txps = PS([128, 128], "tr", 1)
        nc.tensor.matmul(out=txps[:], lhsT=W["rff"][:], rhs=x_fm[0:2, qs], start=True, stop=True)
        t_x = tl.tile([128, 128], F32, tag="t_x", bufs=2)
        nc.vector.tensor_copy(out=t_x[:], in_=txps[:])

        # ---- hoisted RFF features for all 4 chunks; the 16 Sin calls land
        # contiguously in the scalar queue (one trig table load per tile)
        fqs, fvs = [], []
        for c in range(NCH):
            q0 = CQ * c
            qsl = slice(q0, q0 + CQ)
            cs = slice(CR * c, CR * (c + 1))
            # p@Bs from hi/lo split: 6-row bf16 matmul (table rows 32-37)
            pbs = PS([128, CR], "mm", 3)
            nc.tensor.matmul(
                out=pbs[:], lhsT=W["rff6"][32:38, :], rhs=cfm_cs[c][32:38, 1, :],
                start=True, stop=True,
            )
            tfull = ck.tile([128, CQ, K], F32, tag="tfull", bufs=2)
            nc.vector.tensor_tensor(
                out=tfull[:], in0=_bcast_inner(t_x[:, qsl], K),
                in1=pbs[:].rearrange("p (a b) -> p a b", a=CQ), op=OP.subtract,
            )
            ti = ck.tile([128, CQ, K], I32, tag="ti", bufs=2)
            nc.vector.tensor_copy(out=ti[:], in_=tfull[:])
            fs = ck.tile([128, CQ, K], F32, tag="fs", bufs=2)
            nc.vector.tensor_tensor(out=fs[:], in0=tfull[:], in1=ti[:], op=OP.subtract)
            dc0 = ck.tile([128, CQ, K], F32, tag="dc0", bufs=2)
            nc.vector.tensor_scalar(
                out=dc0[:], in0=tfull[:], scalar1=0.25, scalar2=None, op0=OP.add
            )
            ui = ck.tile([128, CQ, K], I32, tag="ui", bufs=2)
            nc.vector.tensor_copy(out=ui[:], in_=dc0[:])
            dc = ck.tile([128, CQ, K], F32, tag="dc", bufs=2)
            nc.vector.tensor_tensor(out=dc[:], in0=dc0[:], in1=ui[:], op=OP.subtract)
            fq = tl.tile([128, CR], BF16, tag=f"fq{c}", bufs=2, name=f"fq{c}_{t}")
            fv = tl.tile([128, CR], BF16, tag=f"fv{c}", bufs=2, name=f"fv{c}_{t}")
            fs2 = fs[:].rearrange("p a b -> p (a b)")
            dc2 = dc[:].rearrange("p a b -> p (a b)")
            nc.scalar.activation(out=fq[0:64, :], in_=fs2[0:64, :], func=AF.Sin, scale=TWO_PI)
            nc.scalar.activation(out=fq[64:128, :], in_=dc2[0:64, :], func=AF.Sin, scale=TWO_PI)
            nc.scalar.activation(out=fv[0:64, :], in_=fs2[64:128, :], func=AF.Sin, scale=TWO_PI)
            nc.scalar.activation(out=fv[64:128, :], in_=dc2[64:128, :], func=AF.Sin, scale=TWO_PI)
            fqs.append(fq)
            fvs.append(fv)

        zacc = [
            tl.tile([128, 128], F32, tag=f"zacc{h}", name=f"zacc{h}_{t}", bufs=2)
            for h in range(H)
        ]
        den_t = tl.tile([128, 128], F32, tag="den_t", bufs=2, name=f"den_{t}")

        # ---------- chunks ----------
        for c in range(NCH):
            q0 = CQ * c
            qsl = slice(q0, q0 + CQ)
            gq = slice(128 * t + q0, 128 * t + q0 + CQ)
            cs = slice(CR * c, CR * (c + 1))
            cg2 = cfm_cs[c][:, 0, :]

            # q path
            g2ps = PS([128, CR], "mm", 3)
            nc.tensor.matmul(out=g2ps[:], lhsT=W["eq_w1"][:], rhs=fqs[c][:], start=True, stop=True)
            g2 = ck.tile([128, CR], BF16, tag="g2", bufs=2)
            nc.scalar.activation(out=g2[:], in_=g2ps[:], func=GELU, bias=W["eq_b1"][:])

            # attention logits in one psum tile, rows {0,32,64,96}.
            # seed = 0.5*gw broadcast to all rows (start=True).
            attps = PS([128, CR], "attps", 2)
            nc.tensor.matmul(
                out=attps[:], lhsT=onesmat_bf[64:65, :],
                rhs=gw_t[64:65, cs], start=True, stop=False,
                skip_group_check=True,
            )
            # w2v^T g2 for all heads [4, CR] + gathered w1v.c, scattered to
            # rows {32h} by one selector matmul
            w24 = PS([128, CR], "tr", 1)
            nc.tensor.matmul(out=w24[0:4, :], lhsT=W["w2v"][:], rhs=g2[:], start=True, stop=True)
            wsum = ck.tile([4, CR], BF16, tag="wsum", bufs=2)
            nc.vector.tensor_tensor(
                out=wsum[:], in0=w24[0:4, :], in1=cfm_cs[c][0:4, 1, :], op=OP.add
            )
            nc.tensor.matmul(
                out=attps[:], lhsT=W["sel4"][:], rhs=wsum[:],
                start=False, stop=False, skip_group_check=True,
            )
            for h in range(H):
                ups = PS([128, CR], "mm", 3)
                nc.tensor.matmul(out=ups[:], lhsT=Wh("Mq", h), rhs=g2[:], start=True, stop=True)
                qkp = ck.tile([128, CR], BF16, tag="qkp", bufs=2)
                nc.vector.tensor_tensor(out=qkp[:], in0=ups[:], in1=cg2, op=OP.mult)
                nc.tensor.matmul(
                    out=attps[32 * h : 32 * h + 1, :], lhsT=ones_col_bf[:], rhs=qkp[:],
                    start=False, stop=(h == H - 1), tile_position=(0, 32 * h),
                    skip_group_check=True,
                )

            # v path
            ev1ps = PS([128, CR], "mm", 3)
            nc.tensor.matmul(out=ev1ps[:], lhsT=W["ev_w1"][:], rhs=fvs[c][:], start=True, stop=True)
            ev1 = ck.tile([128, CR], BF16, tag="ev1", bufs=2)
            nc.scalar.activation(out=ev1[:], in_=ev1ps[:], func=GELU, bias=W["ev_b1"][:])
            ev2ps = PS([128, CR], "mm", 3)
            nc.tensor.matmul(out=ev2ps[:], lhsT=W["ev_w2"][:], rhs=ev1[:], start=True, stop=True)
            mv = ck.tile([128, CQ, K], BF16, tag="mv", bufs=2)
            nc.vector.tensor_tensor(
                out=mv[:], in0=ev2ps[:].rearrange("p (a b) -> p a b", a=CQ),
                in1=_bcast_inner(gp1[:, gq], K), op=OP.mult,
            )
            ivin = ck.tile([128, CQ, K], BF16, tag="ivin", bufs=2)
            nc.vector.tensor_tensor(
                out=ivin[:], in0=mv[:], in1=_bcast_inner(modadd[:, gq], K), op=OP.add
            )
            iv1ps = PS([128, CR], "mm", 3)
            nc.tensor.matmul(
                out=iv1ps[:], lhsT=W["ivw1"][:],
                rhs=ivin[:].rearrange("p a b -> p (a b)"), start=True, stop=True,
            )
            iv1 = ck.tile([128, CR], BF16, tag="iv1", bufs=2)
            nc.scalar.activation(out=iv1[:], in_=iv1ps[:], func=GELU, bias=W["ivb1"][:])

            # iv LN (rstd via ln/exp; the ln/exp block below also carries the
            # previous chunk's deferred mrstd/att_e work)
            ivmean = PS([128, CR], "rows4", 2)
            nc.tensor.matmul(out=ivmean[0:1, :], lhsT=inv128_bf[:], rhs=iv1[:], start=True, stop=True)
            ivsq = ck.tile([128, CR], BF16, tag="ivsq", bufs=2)
            nc.vector.tensor_tensor(out=ivsq[:], in0=iv1[:], in1=iv1[:], op=OP.mult)
            ivmsq = PS([128, CR], "rows4", 2)
            nc.tensor.matmul(out=ivmsq[0:1, :], lhsT=inv128_bf[:], rhs=ivsq[:], start=True, stop=True)
            im2 = ck.tile([1, CR], F32, tag="im2")
            nc.scalar.square(out=im2[:], in_=ivmean[0:1, :])
            ivar = ck.tile([1, CR], F32, tag="ivar")
            nc.vector.tensor_tensor(out=ivar[:], in0=ivmsq[0:1, :], in1=im2[:], op=OP.subtract)

            ilnv = ck.tile([1, CR], F32, tag="ilnv")
            nc.scalar.activation(out=ilnv[:], in_=ivar[:], func=AF.Ln, bias=eps_col[0:1, :])
            irstd_bf = ck.tile([1, CR], BF16, tag="irstd_bf")
            nc.scalar.activation(out=irstd_bf[:], in_=ilnv[:], func=AF.Exp, scale=-0.5)

            # deferred ln/exp + weighted-sum work of the previous chunk,
            # then (entering a new tile) the previous tile's epilogue
            if carry[0] is not None:
                deferred_stage(carry[0])
                carry[0] = None
            if c == 0 and pending_epi[0] is not None:
                emit_epilogue(pending_epi[0])
                pending_epi[0] = None

            imr_bf = ck.tile([1, CR], BF16, tag="imr_bf")
            nc.vector.tensor_tensor(
                out=imr_bf[:], in0=ivmean[0:1, :], in1=irstd_bf[:], op=OP.mult
            )
            irbc = PS([128, CR], "mm", 3)
            nc.tensor.matmul(out=irbc[:], lhsT=ones_row_bf[:], rhs=irstd_bf[:], start=True, stop=True)
            imbc = PS([128, CR], "mm", 3)
            nc.tensor.matmul(out=imbc[:], lhsT=ones_row_bf[:], rhs=imr_bf[:], start=True, stop=True)
            z1 = ck.tile([128, CR], BF16, tag="z1", bufs=2)
            nc.vector.tensor_tensor(out=z1[:], in0=iv1[:], in1=irbc[:], op=OP.mult)
            ziv = ck.tile([128, CR], BF16, tag="ziv", bufs=2)
            nc.vector.tensor_tensor(out=ziv[:], in0=z1[:], in1=imbc[:], op=OP.subtract)

            # per-head v1 -> gm ; m-LN stats to psum rows {32h}.
            # DVE memset seeds the rows the per-head matmuls don't write
            # (their values are never used, but reads must be initialized).
            gms = []
            mmean = PS([128, CR], "rows4", 2)
            msqp = PS([128, CR], "rows4", 2)
            nc.tensor.matmul(
                out=mmean[:], lhsT=zeros_row_bf[:], rhs=zeros_row512_bf[:],
                start=True, stop=True, skip_group_check=True,
            )
            nc.tensor.matmul(
                out=msqp[:], lhsT=zeros_row_bf[:], rhs=zeros_row512_bf[:],
                start=True, stop=True, skip_group_check=True,
            )
            for h in range(H):
                vgps = PS([128, CR], "mm", 3)
                nc.tensor.matmul(out=vgps[:], lhsT=Wh("ivw2g", h), rhs=ziv[:], start=True, stop=True)
                vg_sb = ck.tile([128, CR], BF16, tag="vg_sb", bufs=2)
                nc.scalar.copy(out=vg_sb[:], in_=vgps[:])
                v0ps = PS([128, CR], "mm", 3)
                nc.tensor.matmul(out=v0ps[:], lhsT=Wh("wv", h), rhs=cg2, start=True, stop=True)
                p_sb = ck.tile([128, CR], BF16, tag="p_sb", bufs=2)
                nc.vector.tensor_tensor(out=p_sb[:], in0=v0ps[:], in1=vg_sb[:], op=OP.mult)
                m1ps = PS([128, CR], "mm", 3)
                nc.tensor.matmul(out=m1ps[:], lhsT=W["mw1"][:], rhs=p_sb[:], start=True, stop=False)
                nc.tensor.matmul(out=m1ps[:], lhsT=Wh("WA", h), rhs=cg2, start=False, stop=False)
                nc.tensor.matmul(out=m1ps[:], lhsT=Wh("WB", h), rhs=ziv[:], start=False, stop=True)
                gm = ck.tile([128, CR], BF16, tag=f"gm{h}", bufs=2)
                nc.scalar.activation(out=gm[:], in_=m1ps[:], func=GELU, bias=W["mb1p"][:, h : h + 1])
                gms.append(gm)
                nc.tensor.matmul(
                    out=mmean[32 * h : 32 * h + 1, :], lhsT=inv128_bf[:], rhs=gm[:],
                    start=True, stop=True, tile_position=(0, 32 * h),
                    skip_group_check=True,
                )
                gsq = ck.tile([128, CR], BF16, tag="gsq", bufs=2)
                nc.vector.tensor_tensor(out=gsq[:], in0=gm[:], in1=gm[:], op=OP.mult)
                nc.tensor.matmul(
                    out=msqp[32 * h : 32 * h + 1, :], lhsT=inv128_bf[:], rhs=gsq[:],
                    start=True, stop=True, tile_position=(0, 32 * h),
                    skip_group_check=True,
                )

            carry[0] = (attps, mmean, msqp, gms, qsl, zacc, den_t)

        # epilogue for THIS tile can only be emitted after its chunk-3
        # deferred stage (inside the next tile's first chunk, or right
        # here for the last tile)
        pending_epi[0] = (zacc, den_t, qs)

    deferred_stage(carry[0])
    carry[0] = None
    emit_epilogue(pending_epi[0])
    pending_epi[0] = None

    for p in reversed(_pools):
        p.release()


# ======================= host side =======================


def _host_prep(inputs):
    f = {k: np.asarray(v, np.float32) for k, v in inputs.items()}

    def bf(x):
        return np.ascontiguousarray(np.asarray(x, np.float32)).astype(ml_dtypes.bfloat16)

    def col(x):
        return np.ascontiguousarray(np.asarray(x, np.float32).reshape(-1, 1))

    rff = np.concatenate([FQ * f["rffq"], FV * f["rffv"]], axis=1)  # [2,128]
    # hi/lo split of rff rows for the on-device p@Bs matmul:
    # (r_hi + r_lo)(p_hi + p_lo) ~ r_hi*p_hi + r_hi*p_lo + r_lo*p_hi
    rhi = rff.astype(ml_dtypes.bfloat16).astype(np.float32)
    rlo = rff - rhi
    rff6 = np.zeros((38, 128), np.float32)
    rff6[32] = rhi[0]
    rff6[33] = rhi[0]
    rff6[34] = rlo[0]
    rff6[35] = rhi[1]
    rff6[36] = rhi[1]
    rff6[37] = rlo[1]

    sel4 = np.zeros((4, 128), np.float32)
    for h in range(H):
        sel4[h, 32 * h] = 1.0

    wq_s = f["wq"] * SCALE
    bq_s = f["bq"] * SCALE
    W_qm = f["eq_w2"] @ wq_s
    b_qm = f["eq_b2"] @ wq_s + bq_s
    Mq = np.zeros((128, 512), np.float32)
    w1v = np.zeros((128, 4), np.float32)  # per-head w1v vectors (feature dim)
    w2v = np.zeros((128, 4), np.float32)
    attconst = np.zeros((128, 1), np.float32)
    for h in range(H):
        sl = slice(128 * h, 128 * (h + 1))
        Wq_h = W_qm[:, sl]
        wk_h = f["wk"][:, sl]
        bk_h = f["bk"][sl]
        bq_h = b_qm[sl]
        Mq[:, sl] = Wq_h @ wk_h.T
        w1v[:, h] = wk_h @ bq_h
        w2v[:, h] = Wq_h @ bk_h
        attconst[32 * h, 0] = float(bq_h @ bk_h)

    ivw2f = f["ivls"][:, None] * f["ivw2"]
    ivb2f = f["ivb2"] + f["ivlb"] @ f["ivw2"]
    ivw2g = ivw2f[:, :HD]
    ivw2b = ivw2f[:, HD:]
    # bilinear expansion: m1 = mw1.T (v0*vg) + WA.T cg + WB.T ziv + mb1p
    WA = np.zeros((128, 512), np.float32)
    WB = np.zeros((128, 512), np.float32)
    mb1p = np.zeros((128, H), np.float32)
    for h in range(H):
        sl = slice(128 * h, 128 * (h + 1))
        c1_h = 1.0 + ivb2f[:HD][sl]
        bv_h = f["bv"][sl]
        b2_h = ivb2f[HD:][sl]
        WA[:, sl] = f["wv"][:, sl] @ np.diag(c1_h) @ f["mw1"]
        WB[:, sl] = (ivw2g[:, sl] @ np.diag(bv_h) + ivw2b[:, sl]) @ f["mw1"]
        mb1p[:, h] = f["mb1"] + (bv_h * c1_h + b2_h) @ f["mw1"]

    mw2f = f["mls"][:, None] * f["mw2"]
    mb2f = f["mb2"] + f["mlb"] @ f["mw2"]
    Wmo = np.zeros((128, 512), np.float32)
    for h in range(H):
        wo_h = f["wo"][128 * h : 128 * (h + 1), :]
        Wmo[:, 128 * h : 128 * (h + 1)] = mw2f @ wo_h
    bmo = f["bo"] + sum(mb2f @ f["wo"][128 * h : 128 * (h + 1), :] for h in range(H))

    cw2f = f["cls"][:, None] * f["cw2"]
    cb2f = f["cb2"] + f["clb"] @ f["cw2"]

    weights = {
        "rff": np.ascontiguousarray(rff),
        "rff6": bf(rff6),
        "sel4": bf(sel4),
        "eq_w1": bf(f["eq_w1"]),
        "eq_b1": col(f["eq_b1"]),
        "Mq": bf(Mq),
        "w2v": bf(w2v),
        "attconst": attconst.astype(np.float32),
        "ev_w1": bf(f["ev_w1"]),
        "ev_b1": col(f["ev_b1"]),
        "ev_w2": bf(f["ev_w2"]),
        "ev_b2": col(f["ev_b2"]),
        "ivw1": bf(f["ivw1"]),
        "ivb1": col(f["ivb1"]),
        "ivw2g": bf(ivw2g),
        "wv": bf(f["wv"]),
        "WA": bf(WA),
        "WB": bf(WB),
        "mw1": bf(f["mw1"]),
        "mb1p": np.ascontiguousarray(mb1p),
        "Wmo": bf(Wmo),
        "bmo": col(bmo),
        "cw1": bf(f["cw1"]),
        "cb1": col(f["cb1"]),
        "cw2g": bf(cw2f[:, :DH]),
        "cw2b": bf(cw2f[:, DH:]),
        "cb2g1": col(cb2f[:DH] + 1.0),
        "cb2b": col(cb2f[DH:]),
    }

    x_flat = f["x"].reshape(B * N, CD)
    xh_flat = f["x_h"].reshape(B * N, DH)

    in_maps = []
    for i in range(NCORES):
        b = (i * NQ) // N
        rs = slice(i * NQ, (i + 1) * NQ)
        p_b = f["p"][b]
        c_b = f["c"][b]
        sig_b = f["window_sigma"][b]
        inv2 = 1.0 / (sig_b[:, 0] ** 2)
        phi = p_b.astype(ml_dtypes.bfloat16)
        plo = (p_b - phi.astype(np.float32)).astype(ml_dtypes.bfloat16)
        w1vc = (c_b @ w1v).astype(np.float32)  # [L, 4]
        ctbl = np.zeros((L, 2 * D), ml_dtypes.bfloat16)
        ctbl[:, :D] = bf(c_b)
        ctbl[:, D + 0 : D + 4] = bf(w1vc)
        ctbl[:, D + 32] = phi[:, 0]
        ctbl[:, D + 33] = plo[:, 0]
        ctbl[:, D + 34] = phi[:, 0]
        ctbl[:, D + 35] = phi[:, 1]
        ctbl[:, D + 36] = plo[:, 1]
        ctbl[:, D + 37] = phi[:, 1]
        ctbl[:, D + 64] = (0.5 * inv2).astype(ml_dtypes.bfloat16)
        p2t = np.zeros((3, L), np.float32)
        p2t[0:2] = (2.0 * p_b).T
        p2t[2] = -(p_b**2).sum(1)
        x3 = np.concatenate(
            [x_flat[rs], np.ones((NQ, 1), np.float32)], axis=1
        )
        m = {
            "x": np.ascontiguousarray(x3),
            "xh": np.ascontiguousarray(xh_flat[rs]),
            "ctbl": ctbl,
            "p2t": np.ascontiguousarray(p2t),
        }
        m.update(weights)
        in_maps.append(m)
    return in_maps


_PROGRAM_CACHE = {}


def kernel(**inputs):
    in_maps = _host_prep(inputs)
    if "nc" not in _PROGRAM_CACHE:
        _PROGRAM_CACHE["nc"] = build_program()
    nc = _PROGRAM_CACHE["nc"]

    from concourse.bass_utils import run_bass_kernel_spmd

    res = run_bass_kernel_spmd(nc, in_maps, core_ids=list(range(NCORES)))
    outs = [np.asarray(res.results[i]["out"], np.float32) for i in range(NCORES)]
    return np.concatenate(outs, axis=0).reshape(B, N, DH)


# revision 42
# speedup vs baseline: 1.3060x; 1.3060x over previous
"""EquivariantCrossAttention Trainium2 kernel (8 NeuronCores, SPMD).

kernel(**inputs) takes the FULL unsharded inputs from reference's
setup_inputs() and returns the FULL (B, N, DH) float32 output.

Sharding: flattened query axis (B*N = 4096) split into 8 shards of 512
queries; core i gets queries [512*i, 512*(i+1)) plus its batch's latent
tables. Weights replicated.

Hardcoded problem shapes: B=2 N=2048 L=1024 K=16 CD=2 H=4 DH=128 HD=512.

Algebraic folds done host-side (exact):
  - LayerNorm affines folded into the following Dense weights
  - attention SCALE and eq_w2 folded into wq (W_qm = eq_w2 @ (wq*SCALE))
  - q.k per head via M_h = W_qm_h @ wk_h^T:
      att = g2^T M_h cg + g2.w2v_h + cg.w1v_h + const_h
    The cg.w1v_h term is precomputed per latent on host and gathered.
  - mFFN dense2 and wo merged (W_mo_h = mw2' @ wo_h) and moved after the
    attention sum (softmax weights sum to 1; dense2 affine)
  - mFFN LN normalization folded into attention weights:
      sum_k att*LN(g) = sum_k (att*rstd) g - sum_k att*mean*rstd
  - RFF: t = x@Bs - p@Bs; p@Bs computed on device from hi/lo bf16 split
    of p (3-term product vs hi/lo split of Bs rows);
    sin(2pi t) = Sin(2pi(t - rint t)); cos via +0.25 shift.
  - per-latent gather table row (256 bf16 = 512B):
      [ c (128) | w1vc_h (4) | p0hi p0lo p0hi p1hi p1lo p1hi |
        1/sigma^2 | pad ]
    gathered once per query tile (2048 idxs) with transpose=True.
  - rstd = exp(-0.5*ln(var+eps)) so LN rstds share the ln/exp ACT table
    set with the softmax Exp; mrstd/att_e of chunk c are emitted in
    chunk c+1's ln/exp block (software pipelining) so the scalar engine
    switches table sets only ~2.5x per chunk.

Device structure per core: 4 query tiles x 128 queries; per tile:
scores via PE (|p|^2 folded in via a ones-row) -> top-16 via DVE
max/max_index/match_replace -> one transposed dma_gather -> hoisted
RFF/sin features for all 4 chunks -> 4 chunks of 512 rows (32 queries x
16 neighbors, q-major) through the fused MLP/attention pipeline.
"""

import sys

sys.path.insert(0, "/opt/trn_rl_repo")

import numpy as np
import ml_dtypes

import concourse.bass as bass
import concourse.bacc as bacc
import concourse.mybir as mybir
import concourse.tile as tile
from concourse.masks import make_identity

F32 = mybir.dt.float32
BF16 = mybir.dt.bfloat16
U32 = mybir.dt.uint32
I32 = mybir.dt.int32
I16 = mybir.dt.int16
AF = mybir.ActivationFunctionType
OP = mybir.AluOpType
AX = mybir.AxisListType

B, N, L, K, CD, H, DH, D = 2, 2048, 1024, 16, 2, 4, 128, 128
HD = H * DH
FQ = 2.0
FV = 2.0
SCALE = 1.0 / float(np.sqrt(DH))
NCORES = 8
NQ = (B * N) // NCORES  # queries per core = 512
QT = NQ // 128  # query tiles per core = 4
NCH = 4  # chunks per query tile
CQ = 128 // NCH  # queries per chunk = 32
CR = CQ * K  # rows per chunk = 512
TR = 128 * K  # rows per tile = 2048
GELU = AF.Gelu_apprx_tanh
TWO_PI = 2.0 * np.pi
EPS = 1e-6

WSPECS = [
    ("rff", [CD, 128], F32),
    ("rff6", [38, 128], BF16),
    ("sel4", [4, 128], BF16),
    ("eq_w1", [128, 128], BF16),
    ("eq_b1", [128, 1], F32),
    ("Mq", [128, 512], BF16),
    ("w2v", [128, 4], BF16),
    ("attconst", [128, 1], F32),
    ("ev_w1", [128, 128], BF16),
    ("ev_b1", [128, 1], F32),
    ("ev_w2", [128, 128], BF16),
    ("ev_b2", [128, 1], F32),
    ("ivw1", [128, 128], BF16),
    ("ivb1", [128, 1], F32),
    ("ivw2g", [128, 512], BF16),
    ("wv", [128, 512], BF16),
    ("WA", [128, 512], BF16),
    ("WB", [128, 512], BF16),
    ("mw1", [128, 128], BF16),
    ("mb1p", [128, 4], F32),
    ("Wmo", [128, 512], BF16),
    ("bmo", [128, 1], F32),
    ("cw1", [128, 128], BF16),
    ("cb1", [128, 1], F32),
    ("cw2g", [128, 128], BF16),
    ("cw2b", [128, 128], BF16),
    ("cb2g1", [128, 1], F32),
    ("cb2b", [128, 1], F32),
]


def _bcast_inner(ap, n):
    """[.., Q] AP -> [.., Q, n] with a stride-0 inner dim (free broadcast)."""
    newap = [list(p) for p in ap.ap] + [[0, n]]
    return bass.AP(ap.tensor, ap.offset, newap)


def build_program():
    nc = bacc.Bacc()

    x_d = nc.declare_dram_parameter("x", [NQ, 3], F32, isOutput=False)
    xh_d = nc.declare_dram_parameter("xh", [NQ, DH], F32, isOutput=False)
    ctbl_d = nc.declare_dram_parameter("ctbl", [L, 2 * D], BF16, isOutput=False)
    p2t_d = nc.declare_dram_parameter("p2t", [3, L], F32, isOutput=False)
    w_d = {}
    for name, shape, dt in WSPECS:
        w_d[name] = nc.declare_dram_parameter(name, shape, dt, isOutput=False)
    out_d = nc.declare_dram_parameter("out", [NQ, DH], F32, isOutput=True)

    with tile.TileContext(nc) as tc:
        _emit(nc, tc, x_d, xh_d, ctbl_d, p2t_d, w_d, out_d)
    nc.compile()
    _optimize_act_table_loads(nc)
    return nc


def _optimize_act_table_loads(nc):
    """Remap ln-only/exp-only ACT table-set loads to the combined
    natural_log_exp_and_others set, then drop consecutive reloads of an
    already-resident set. The stock placement picks the first set
    containing each function, which costs a ~1.5us table DMA at every
    Ln<->Exp alternation in the LN-rstd / softmax-exp blocks."""
    from concourse.hw_specs import get_activation_tables

    names = list(get_activation_tables(nc.m.arch).keys())
    try:
        ln_id = names.index("natural_log")
        exp_id = names.index("exp_and_others")
        combo_id = names.index("natural_log_exp_and_others")
    except ValueError:
        return
    for fn in nc.m.functions:
        for blk in fn.blocks:
            dead = []
            cur = {}
            for inst in blk.instructions:
                if isinstance(inst, mybir.InstLoadActFuncSet):
                    if inst.act_func_set_id in (ln_id, exp_id):
                        inst.act_func_set_id = combo_id
                    e = str(inst.engine)
                    si = inst.sync_info
                    clean = si is None or (
                        len(si.on_wait) == 0 and len(si.on_update) == 0
                    )
                    if cur.get(e) == inst.act_func_set_id and clean:
                        dead.append(inst)
                    else:
                        cur[e] = inst.act_func_set_id
            for inst in dead:
                blk.instructions.remove(inst)


def _emit(nc, tc, x_d, xh_d, ctbl_d, p2t_d, w_d, out_d):
    const = tc.alloc_tile_pool(name="const", bufs=1)
    wpool = tc.alloc_tile_pool(name="wpool", bufs=1)
    core = tc.alloc_tile_pool(name="core", bufs=1)
    tl = tc.alloc_tile_pool(name="tl", bufs=1)
    ck = tc.alloc_tile_pool(name="ck", bufs=1)
    psp = tc.alloc_tile_pool(name="psp", bufs=1, space="PSUM")
    _pools = [const, wpool, core, tl, ck, psp]

    _psn = [0]

    def PS(shape, tag, bufs, dtype=F32):
        _psn[0] += 1
        return psp.tile(
            shape, dtype, space="PSUM", tag=tag, bufs=bufs, name=f"ps_{tag}_{_psn[0]}"
        )

    # ---------- constants ----------
    ident = const.tile([128, 128], F32)
    make_identity(nc, ident[:])

    ones_col_bf = const.tile([128, 1], BF16)
    nc.vector.memset(ones_col_bf[:], 1.0)
    inv128_bf = const.tile([128, 1], BF16)
    nc.vector.memset(inv128_bf[:], 1.0 / 128.0)
    ones_row_bf = const.tile([1, 128], BF16)
    nc.vector.memset(ones_row_bf[:], 1.0)
    onesmat_bf = const.tile([128, 128], BF16)
    nc.vector.memset(onesmat_bf[:], 1.0)
    eps_col = const.tile([128, 1], F32)
    nc.vector.memset(eps_col[:], EPS)
    zeros_row_bf = const.tile([1, 128], BF16)
    nc.vector.memset(zeros_row_bf[:], 0.0)
    zeros_row512_bf = const.tile([1, 512], BF16)
    nc.vector.memset(zeros_row512_bf[:], 0.0)

    # ---------- weights ----------
    W = {}
    for name, shape, dt in WSPECS:
        wt = wpool.tile(shape, dt, name=f"w_{name}", tag=f"w_{name}")
        nc.sync.dma_start(out=wt[:], in_=w_d[name][:])
        W[name] = wt

    def Wh(name, h, w=128):
        return W[name][:, h * w : (h + 1) * w]

    # ---------- per-core precompute ----------
    # x arrives with a ones column appended (for the |p|^2 fold in scores)
    x_sb = core.tile([128, QT, 3], F32)
    nc.sync.dma_start(out=x_sb[:], in_=x_d[:].rearrange("(t q) c -> q t c", q=128))
    xsq = core.tile([128, QT], F32)
    xs2 = core.tile([128, QT, CD], F32)
    nc.vector.tensor_tensor(
        out=xs2[:], in0=x_sb[:, :, 0:CD], in1=x_sb[:, :, 0:CD], op=OP.mult
    )
    nc.vector.tensor_reduce(out=xsq[:], in_=xs2[:], axis=AX.X, op=OP.add)

    x_fm = core.tile([3, NQ], F32)
    for t in range(QT):
        tp = PS([3, 128], "tr", 1)
        nc.tensor.transpose(out=tp[:], in_=x_sb[:, t, :], identity=ident[:])
        nc.vector.tensor_copy(out=x_fm[:, 128 * t : 128 * (t + 1)], in_=tp[:])

    p2_fm = core.tile([3, L], F32)
    nc.sync.dma_start(out=p2_fm[:], in_=p2t_d[:])

    # ---------- cFFN on x_h (512 queries at once) ----------
    xh_fm = core.tile([128, NQ], BF16)
    xh_rm = core.tile([128, QT, DH], F32)
    nc.sync.dma_start(out=xh_rm[:], in_=xh_d[:].rearrange("(t q) c -> q t c", q=128))
    for t in range(QT):
        tp = PS([128, 128], "tr", 1)
        nc.tensor.transpose(out=tp[:], in_=xh_rm[:, t, :], identity=ident[:])
        nc.vector.tensor_copy(out=xh_fm[:, 128 * t : 128 * (t + 1)], in_=tp[:])

    c1ps = PS([128, NQ], "mm", 3)
    nc.tensor.matmul(out=c1ps[:], lhsT=W["cw1"][:], rhs=xh_fm[:], start=True, stop=True)
    c1 = core.tile([128, NQ], BF16)
    nc.scalar.activation(out=c1[:], in_=c1ps[:], func=GELU, bias=W["cb1"][:])

    cmean = PS([128, NQ], "rows4", 2)
    nc.tensor.matmul(out=cmean[0:1, :], lhsT=inv128_bf[:], rhs=c1[:], start=True, stop=True)
    c1sq = core.tile([128, NQ], BF16)
    nc.vector.tensor_tensor(out=c1sq[:], in0=c1[:], in1=c1[:], op=OP.mult)
    cmsq = PS([128, NQ], "rows4", 2)
    nc.tensor.matmul(out=cmsq[0:1, :], lhsT=inv128_bf[:], rhs=c1sq[:], start=True, stop=True)

    cm2 = core.tile([1, NQ], F32)
    nc.scalar.square(out=cm2[:], in_=cmean[0:1, :])
    cvar = core.tile([1, NQ], F32)
    nc.vector.tensor_tensor(out=cvar[:], in0=cmsq[0:1, :], in1=cm2[:], op=OP.subtract)
    # rstd = exp(-0.5*ln(var+eps)); stays in the ln/exp table set
    clnv = core.tile([1, NQ], F32)
    nc.scalar.activation(out=clnv[:], in_=cvar[:], func=AF.Ln, bias=eps_col[0:1, :])
    crstd_bf = core.tile([1, NQ], BF16)
    nc.scalar.activation(out=crstd_bf[:], in_=clnv[:], func=AF.Exp, scale=-0.5)
    cmr_bf = core.tile([1, NQ], BF16)
    nc.vector.tensor_tensor(out=cmr_bf[:], in0=cmean[0:1, :], in1=crstd_bf[:], op=OP.mult)
    crbc = PS([128, NQ], "mm", 3)
    nc.tensor.matmul(out=crbc[:], lhsT=ones_row_bf[:], rhs=crstd_bf[:], start=True, stop=True)
    cmbc = PS([128, NQ], "mm", 3)
    nc.tensor.matmul(out=cmbc[:], lhsT=ones_row_bf[:], rhs=cmr_bf[:], start=True, stop=True)
    z1c = core.tile([128, NQ], BF16)
    nc.vector.tensor_tensor(out=z1c[:], in0=c1[:], in1=crbc[:], op=OP.mult)
    zc = core.tile([128, NQ], BF16)
    nc.vector.tensor_tensor(out=zc[:], in0=z1c[:], in1=cmbc[:], op=OP.subtract)

    gp1 = core.tile([128, NQ], BF16)
    modadd = core.tile([128, NQ], BF16)
    gps_ = PS([128, NQ], "mm", 3)
    nc.tensor.matmul(out=gps_[:], lhsT=W["cw2g"][:], rhs=zc[:], start=True, stop=True)
    nc.scalar.activation(out=gp1[:], in_=gps_[:], func=AF.Identity, bias=W["cb2g1"][:])
    btps = PS([128, NQ], "mm", 3)
    nc.tensor.matmul(out=btps[:], lhsT=W["cw2b"][:], rhs=zc[:], start=True, stop=True)
    bt = core.tile([128, NQ], BF16)
    nc.scalar.activation(out=bt[:], in_=btps[:], func=AF.Identity, bias=W["cb2b"][:])
    ma1 = core.tile([128, NQ], BF16)
    nc.vector.tensor_scalar(
        out=ma1[:], in0=gp1[:], scalar1=W["ev_b2"][:], scalar2=None, op0=OP.mult
    )
    nc.vector.tensor_tensor(out=modadd[:], in0=ma1[:], in1=bt[:], op=OP.add)

    # deferred-stage state carried across chunks: (attps, mmean, msqp, gms,
    # qsl, zacc, den_t); tile epilogues are likewise deferred until after
    # the tile's chunk-3 deferred stage has been emitted.
    carry = [None]
    pending_epi = [None]

    def deferred_stage(state):
        """ln/exp block + weighted-sum DVE work for a completed chunk.

        Emitted one chunk later so mrstd/att_e share the ln/exp ACT table
        set with the next chunk's irstd."""
        attps, mmean, msqp, gms, qsl, zacc, den_t = state

        # mm2/mvar/mrstd: only rows {0,32,64,96} carry real stats; other
        # rows hold stale/garbage psum values whose results are never read.
        mm2 = ck.tile([128, CR], F32, tag="mm2")
        nc.scalar.square(out=mm2[:], in_=mmean[:])
        mvar = ck.tile([128, CR], F32, tag="mvar")
        nc.vector.tensor_tensor(out=mvar[:], in0=msqp[:], in1=mm2[:], op=OP.subtract)
        mlnv = ck.tile([128, CR], F32, tag="mlnv")
        nc.scalar.activation(out=mlnv[:], in_=mvar[:], func=AF.Ln, bias=eps_col[:])
        mrstd = ck.tile([128, CR], BF16, tag="mrstd")
        nc.scalar.activation(out=mrstd[:], in_=mlnv[:], func=AF.Exp, scale=-0.5)

        att_e = ck.tile([128, CR], BF16, tag="att_e")
        nc.scalar.activation(out=att_e[:], in_=attps[:], func=AF.Exp, bias=W["attconst"][:])
        nc.vector.tensor_reduce(
            out=den_t[:, qsl], in_=att_e[:].rearrange("p (a b) -> p a b", a=CQ),
            axis=AX.X, op=OP.add,
        )

        a2 = ck.tile([128, CR], BF16, tag="a2")
        nc.vector.tensor_tensor(out=a2[:], in0=att_e[:], in1=mrstd[:], op=OP.mult)
        # a3 = att_e * mean * rstd = a2 * mean  (psum operand)
        a3 = ck.tile([128, CR], BF16, tag="a3")
        nc.vector.tensor_tensor(out=a3[:], in0=a2[:], in1=mmean[:], op=OP.mult)
        s3bf = ck.tile([128, CQ], BF16, tag="s3bf")
        with nc.allow_low_precision(reason="16-term bf16 row sum; folded into bf16 matmul anyway"):
            nc.vector.tensor_reduce(
                out=s3bf[:], in_=a3[:].rearrange("p (a b) -> p a b", a=CQ), axis=AX.X,
                op=OP.add,
            )

        for h in range(H):
            a2bc = PS([128, CR], "mm", 3)
            nc.tensor.matmul(
                out=a2bc[:], lhsT=onesmat_bf[32 * h : 32 * h + 1, :],
                rhs=a2[32 * h : 32 * h + 1, :], start=True, stop=True,
                tile_position=(32 * h, 0),
            )
            zp = ck.tile([128, CR], BF16, tag="zp", bufs=2)
            nc.vector.tensor_tensor(out=zp[:], in0=gms[h][:], in1=a2bc[:], op=OP.mult)
            nc.vector.tensor_reduce(
                out=zacc[h][:, qsl], in_=zp[:].rearrange("p (a b) -> p a b", a=CQ),
                axis=AX.X, op=OP.add,
            )
            s3bc = PS([128, CQ], "tr", 1)
            nc.tensor.matmul(
                out=s3bc[:], lhsT=onesmat_bf[32 * h : 32 * h + 1, :],
                rhs=s3bf[32 * h : 32 * h + 1, :], start=True, stop=True,
                tile_position=(32 * h, 0),
            )
            nc.vector.tensor_tensor(
                out=zacc[h][:, qsl], in0=zacc[h][:, qsl], in1=s3bc[:], op=OP.subtract
            )

    def emit_epilogue(state):
        """Softmax denominator fold + output matmul/transpose/store for a
        finished tile (emitted after the tile's chunk-3 deferred stage)."""
        zacc, den_t, qs = state
        rden_t = tl.tile([128, 128], F32, tag="rden_t", bufs=2)
        nc.vector.reciprocal(out=rden_t[:], in_=den_t[:])
        rdbf = tl.tile([128, 128], BF16, tag="rdbf", bufs=2)
        nc.vector.tensor_copy(out=rdbf[:], in_=rden_t[:])
        for h in range(H):
            rdbc = PS([128, 128], "tr", 1)
            nc.tensor.matmul(
                out=rdbc[:], lhsT=onesmat_bf[32 * h : 32 * h + 1, :],
                rhs=rdbf[32 * h : 32 * h + 1, :], start=True, stop=True,
                tile_position=(32 * h, 0),
            )
            nc.vector.tensor_tensor(out=zacc[h][:], in0=zacc[h][:], in1=rdbc[:], op=OP.mult)

        outps = PS([128, 128], "tr", 1)
        for h in range(H):
            zbf = tl.tile([128, 128], BF16, tag="zbf")
            nc.vector.tensor_copy(out=zbf[:], in_=zacc[h][:])
            nc.tensor.matmul(
                out=outps[:], lhsT=Wh("Wmo", h), rhs=zbf[:], start=(h == 0), stop=(h == H - 1)
            )
        outsb = tl.tile([128, 128], F32, tag="outsb")
        nc.scalar.activation(out=outsb[:], in_=outps[:], func=AF.Identity, bias=W["bmo"][:])
        trp = PS([128, 128], "tr", 1)
        nc.tensor.transpose(out=trp[:], in_=outsb[:], identity=ident[:])
        outrm = tl.tile([128, 128], F32, tag="outrm")
        nc.vector.tensor_copy(out=outrm[:], in_=trp[:])
        nc.sync.dma_start(out=out_d[qs, :], in_=outrm[:])

    # ---------- prologue: scores -> top-16 -> gathers for ALL tiles ----
    # Emitting every tile's gather chain up front queues all 16 gathers on
    # the gpsimd engine from the start, so no tile boundary ever stalls
    # waiting for its gather (the chunk pipeline consumes gather #k around
    # t~60k us while it completes around t~11k us).
    cfm_all = []
    gw_all = []
    for t in range(QT):
        qs = slice(128 * t, 128 * (t + 1))

        # scores (two 512-wide halves; |p|^2 folded via ones row) + top-16
        scores = tl.tile([128, L], F32, tag="scores")
        for s in range(2):
            sl = slice(512 * s, 512 * (s + 1))
            scps = PS([128, 512], "mm", 3)
            nc.tensor.matmul(out=scps[:], lhsT=x_fm[:, qs], rhs=p2_fm[:, sl], start=True, stop=True)
            nc.scalar.copy(out=scores[:, sl], in_=scps[:])
        vals = tl.tile([128, K], F32, tag="vals", bufs=2)
        idxs = tl.tile([128, K], U32, tag="idxs", bufs=2)
        scr2 = tl.tile([128, L], F32, tag="scr2")
        nc.vector.max(out=vals[:, 0:8], in_=scores[:])
        nc.vector.max_index(out=idxs[:, 0:8], in_max=vals[:, 0:8], in_values=scores[:])
        nc.vector.match_replace(
            out=scr2[:], in_to_replace=vals[:, 0:8], in_values=scores[:], imm_value=-1e30
        )
        nc.vector.max(out=vals[:, 8:16], in_=scr2[:])
        nc.vector.max_index(out=idxs[:, 8:16], in_max=vals[:, 8:16], in_values=scr2[:])

        # index prep for dma_gather: idx16[k, q] (int16) replicated across
        # the 8 gpsimd cores' 16-partition blocks
        idxf = tl.tile([128, K], F32, tag="idxf", bufs=2)
        nc.vector.tensor_copy(out=idxf[:], in_=idxs[:])
        idxt_ps = PS([K, 128], "tr", 1)
        nc.tensor.transpose(out=idxt_ps[:], in_=idxf[:], identity=ident[:])
        idx16 = tl.tile([128, 128], I16, tag="idx16", bufs=QT)
        nc.vector.tensor_copy(out=idx16[0:16, :], in_=idxt_ps[:])
        for b in range(1, 8):
            nc.sync.dma_start(out=idx16[16 * b : 16 * b + 16, :], in_=idx16[0:16, :])

        # transposed gathers (512B rows): cfm [128, 2, 512] per chunk:
        # block 0 = c features; block 1 rows 0-3 = w1v.c per head, rows
        # 32-37 = p hi/lo splits, row 64 = 0.5/sigma^2 (row choices keep
        # every consumer's partition bases legal/aligned)
        cfm_cs = []
        for gc in range(NCH):
            cfm_c = tl.tile([128, 2, CR], BF16, tag=f"cfm{gc}", bufs=3,
                            name=f"cfm{gc}_{t}")
            nc.gpsimd.dma_gather(
                out_ap=cfm_c[:], in_ap=ctbl_d[:],
                idxs_ap=idx16[:, 32 * gc : 32 * gc + 32], num_idxs=CR,
                num_idxs_reg=CR, elem_size=2 * D, transpose=True,
            )
            cfm_cs.append(cfm_c)
        cfm_all.append(cfm_cs)

        # -d^2 (q-major, [128, K]); flattened to a row in the main loop
        negd2 = tl.tile([128, K], BF16, tag="negd2", bufs=QT)
        nc.vector.tensor_scalar(
            out=negd2[:], in0=vals[:], scalar1=xsq[:, t : t + 1], scalar2=None,
            op0=OP.subtract,
        )
        gw_all.append(negd2)

    # ---------- per query tile ----------
    for t in range(QT):
        qs = slice(128 * t, 128 * (t + 1))
        cfm_cs = cfm_all[t]
        negd2 = gw_all[t]

        # -d^2 row DMA-flattened INTO partition 64 so the gw multiply is
        # partition-base-aligned with the gathered 0.5/sigma^2 row.
        # Emitted here, not in the prologue, so the in-order DVE queue
        # never blocks on a still-running gather.
        negd2_row = tl.tile([65, TR], BF16, tag="negd2_row", bufs=2)
        nc.sync.dma_start(out=negd2_row[64:65, :], in_=negd2[:])
        gw_t = tl.tile([65, TR], BF16, tag="gw_t", bufs=2)
        for gc in range(NCH):
            nc.vector.tensor_tensor(
                out=gw_t[64:65, CR * gc : CR * (gc + 1)],
                in0=negd2_row[64:65, CR * gc : CR * (gc + 1)],
                in1=cfm_cs[gc][64:65, 1, :], op=OP.mult,
            )

        # t_x = x @ Bs for this tile [128, 128]
        txps = PS([128, 128], "tr", 1)
        nc.tensor.matmul(out=txps[:], lhsT=W["rff"][:], rhs=x_fm[0:2, qs], start=True, stop=True)
        t_x = tl.tile([128, 128], F32, tag="t_x", bufs=2)
        nc.vector.tensor_copy(out=t_x[:], in_=txps[:])

        # ---- hoisted RFF features for all 4 chunks; the 16 Sin calls land
        # contiguously in the scalar queue (one trig table load per tile)
        fqs, fvs = [], []
        for c in range(NCH):
            q0 = CQ * c
            qsl = slice(q0, q0 + CQ)
            cs = slice(CR * c, CR * (c + 1))
            # p@Bs from hi/lo split: 6-row bf16 matmul (table rows 32-37)
            pbs = PS([128, CR], "mm", 3)
            nc.tensor.matmul(
                out=pbs[:], lhsT=W["rff6"][32:38, :], rhs=cfm_cs[c][32:38, 1, :],
                start=True, stop=True,
            )
            # frac chain; paired ops share a tag (lifetimes don't overlap)
            tfull = ck.tile([128, CQ, K], F32, tag="tfrac", bufs=2)
            nc.vector.tensor_tensor(
                out=tfull[:], in0=_bcast_inner(t_x[:, qsl], K),
                in1=pbs[:].rearrange("p (a b) -> p a b", a=CQ), op=OP.subtract,
            )
            ti = ck.tile([128, CQ, K], I32, tag="ifrac", bufs=2)
            nc.vector.tensor_copy(out=ti[:], in_=tfull[:])
            fs = ck.tile([128, CQ, K], F32, tag="sfrac", bufs=2)
            nc.vector.tensor_tensor(out=fs[:], in0=tfull[:], in1=ti[:], op=OP.subtract)
            dc0 = ck.tile([128, CQ, K], F32, tag="tfrac", bufs=2)
            nc.vector.tensor_scalar(
                out=dc0[:], in0=tfull[:], scalar1=0.25, scalar2=None, op0=OP.add
            )
            ui = ck.tile([128, CQ, K], I32, tag="ifrac", bufs=2)
            nc.vector.tensor_copy(out=ui[:], in_=dc0[:])
            dc = ck.tile([128, CQ, K], F32, tag="sfrac", bufs=2)
            nc.vector.tensor_tensor(out=dc[:], in0=dc0[:], in1=ui[:], op=OP.subtract)
            fq = tl.tile([128, CR], BF16, tag=f"fq{c}", bufs=2, name=f"fq{c}_{t}")
            fv = tl.tile([128, CR], BF16, tag=f"fv{c}", bufs=2, name=f"fv{c}_{t}")
            fs2 = fs[:].rearrange("p a b -> p (a b)")
            dc2 = dc[:].rearrange("p a b -> p (a b)")
            nc.scalar.activation(out=fq[0:64, :], in_=fs2[0:64, :], func=AF.Sin, scale=TWO_PI)
            nc.scalar.activation(out=fq[64:128, :], in_=dc2[0:64, :], func=AF.Sin, scale=TWO_PI)
            nc.scalar.activation(out=fv[0:64, :], in_=fs2[64:128, :], func=AF.Sin, scale=TWO_PI)
            nc.scalar.activation(out=fv[64:128, :], in_=dc2[64:128, :], func=AF.Sin, scale=TWO_PI)
            fqs.append(fq)
            fvs.append(fv)

        zacc = [
            tl.tile([128, 128], F32, tag=f"zacc{h}", name=f"zacc{h}_{t}", bufs=2)
            for h in range(H)
        ]
        den_t = tl.tile([128, 128], F32, tag="den_t", bufs=2, name=f"den_{t}")

        # ---------- chunks ----------
        for c in range(NCH):
            q0 = CQ * c
            qsl = slice(q0, q0 + CQ)
            gq = slice(128 * t + q0, 128 * t + q0 + CQ)
            cs = slice(CR * c, CR * (c + 1))
            cg2 = cfm_cs[c][:, 0, :]

            # v0 = wv^T c for all heads, hoisted off the per-head critical
            # chain (only needs the gather); ACT copies to bf16 so the
            # per-head p_sb multiply can read vgps straight from PSUM
            v0sbs = []
            for h in range(H):
                v0ps = PS([128, CR], "mm", 3)
                nc.tensor.matmul(out=v0ps[:], lhsT=Wh("wv", h), rhs=cg2, start=True, stop=True)
                v0sb = ck.tile([128, CR], BF16, tag=f"v0sb{h}", bufs=2)
                nc.scalar.copy(out=v0sb[:], in_=v0ps[:])
                v0sbs.append(v0sb)

            # q path
            g2ps = PS([128, CR], "mm", 3)
            nc.tensor.matmul(out=g2ps[:], lhsT=W["eq_w1"][:], rhs=fqs[c][:], start=True, stop=True)
            g2 = ck.tile([128, CR], BF16, tag="g2", bufs=2)
            nc.scalar.activation(out=g2[:], in_=g2ps[:], func=GELU, bias=W["eq_b1"][:])

            # attention logits in one psum tile, rows {0,32,64,96}.
            # seed = 0.5*gw broadcast to all rows (start=True).
            attps = PS([128, CR], "attps", 2)
            nc.tensor.matmul(
                out=attps[:], lhsT=onesmat_bf[64:65, :],
                rhs=gw_t[64:65, cs], start=True, stop=False,
                skip_group_check=True,
            )
            # w2v^T g2 for all heads [4, CR] + gathered w1v.c, scattered to
            # rows {32h} by one selector matmul
            w24 = PS([128, CR], "tr", 1)
            nc.tensor.matmul(out=w24[0:4, :], lhsT=W["w2v"][:], rhs=g2[:], start=True, stop=True)
            wsum = ck.tile([4, CR], BF16, tag="wsum", bufs=2)
            nc.vector.tensor_tensor(
                out=wsum[:], in0=w24[0:4, :], in1=cfm_cs[c][0:4, 1, :], op=OP.add
            )
            nc.tensor.matmul(
                out=attps[:], lhsT=W["sel4"][:], rhs=wsum[:],
                start=False, stop=False, skip_group_check=True,
            )
            for h in range(H):
                ups = PS([128, CR], "mm", 3)
                nc.tensor.matmul(out=ups[:], lhsT=Wh("Mq", h), rhs=g2[:], start=True, stop=True)
                qkp = ck.tile([128, CR], BF16, tag="qkp", bufs=2)
                nc.vector.tensor_tensor(out=qkp[:], in0=ups[:], in1=cg2, op=OP.mult)
                nc.tensor.matmul(
                    out=attps[32 * h : 32 * h + 1, :], lhsT=ones_col_bf[:], rhs=qkp[:],
                    start=False, stop=(h == H - 1), tile_position=(0, 32 * h),
                    skip_group_check=True,
                )

            # v path
            ev1ps = PS([128, CR], "mm", 3)
            nc.tensor.matmul(out=ev1ps[:], lhsT=W["ev_w1"][:], rhs=fvs[c][:], start=True, stop=True)
            ev1 = ck.tile([128, CR], BF16, tag="ev1", bufs=2)
            nc.scalar.activation(out=ev1[:], in_=ev1ps[:], func=GELU, bias=W["ev_b1"][:])
            ev2ps = PS([128, CR], "mm", 3)
            nc.tensor.matmul(out=ev2ps[:], lhsT=W["ev_w2"][:], rhs=ev1[:], start=True, stop=True)
            mv = ck.tile([128, CQ, K], BF16, tag="mv", bufs=2)
            nc.vector.tensor_tensor(
                out=mv[:], in0=ev2ps[:].rearrange("p (a b) -> p a b", a=CQ),
                in1=_bcast_inner(gp1[:, gq], K), op=OP.mult,
            )
            ivin = ck.tile([128, CQ, K], BF16, tag="ivin", bufs=2)
            nc.vector.tensor_tensor(
                out=ivin[:], in0=mv[:], in1=_bcast_inner(modadd[:, gq], K), op=OP.add
            )
            iv1ps = PS([128, CR], "mm", 3)
            nc.tensor.matmul(
                out=iv1ps[:], lhsT=W["ivw1"][:],
                rhs=ivin[:].rearrange("p a b -> p (a b)"), start=True, stop=True,
            )
            iv1 = ck.tile([128, CR], BF16, tag="iv1", bufs=2)
            nc.scalar.activation(out=iv1[:], in_=iv1ps[:], func=GELU, bias=W["ivb1"][:])

            # iv LN (rstd via ln/exp; the ln/exp block below also carries the
            # previous chunk's deferred mrstd/att_e work)
            ivmean = PS([128, CR], "rows4", 2)
            nc.tensor.matmul(out=ivmean[0:1, :], lhsT=inv128_bf[:], rhs=iv1[:], start=True, stop=True)
            ivsq = ck.tile([128, CR], BF16, tag="ivsq", bufs=2)
            nc.vector.tensor_tensor(out=ivsq[:], in0=iv1[:], in1=iv1[:], op=OP.mult)
            ivmsq = PS([128, CR], "rows4", 2)
            nc.tensor.matmul(out=ivmsq[0:1, :], lhsT=inv128_bf[:], rhs=ivsq[:], start=True, stop=True)
            im2 = ck.tile([1, CR], F32, tag="im2")
            nc.scalar.square(out=im2[:], in_=ivmean[0:1, :])
            ivar = ck.tile([1, CR], F32, tag="ivar")
            nc.vector.tensor_tensor(out=ivar[:], in0=ivmsq[0:1, :], in1=im2[:], op=OP.subtract)

            ilnv = ck.tile([1, CR], F32, tag="ilnv")
            nc.scalar.activation(out=ilnv[:], in_=ivar[:], func=AF.Ln, bias=eps_col[0:1, :])
            irstd_bf = ck.tile([1, CR], BF16, tag="irstd_bf")
            nc.scalar.activation(out=irstd_bf[:], in_=ilnv[:], func=AF.Exp, scale=-0.5)

            # deferred ln/exp + weighted-sum work of the previous chunk,
            # then (entering a new tile) the previous tile's epilogue
            if carry[0] is not None:
                deferred_stage(carry[0])
                carry[0] = None
            if c == 0 and pending_epi[0] is not None:
                emit_epilogue(pending_epi[0])
                pending_epi[0] = None

            imr_bf = ck.tile([1, CR], BF16, tag="imr_bf")
            nc.vector.tensor_tensor(
                out=imr_bf[:], in0=ivmean[0:1, :], in1=irstd_bf[:], op=OP.mult
            )
            irbc = PS([128, CR], "mm", 3)
            nc.tensor.matmul(out=irbc[:], lhsT=ones_row_bf[:], rhs=irstd_bf[:], start=True, stop=True)
            imbc = PS([128, CR], "mm", 3)
            nc.tensor.matmul(out=imbc[:], lhsT=ones_row_bf[:], rhs=imr_bf[:], start=True, stop=True)
            z1 = ck.tile([128, CR], BF16, tag="z1", bufs=2)
            nc.vector.tensor_tensor(out=z1[:], in0=iv1[:], in1=irbc[:], op=OP.mult)
            ziv = ck.tile([128, CR], BF16, tag="ziv", bufs=2)
            nc.vector.tensor_tensor(out=ziv[:], in0=z1[:], in1=imbc[:], op=OP.subtract)

            # per-head v1 -> gm ; m-LN stats to psum rows {32h}.
            # DVE memset seeds the rows the per-head matmuls don't write
            # (their values are never used, but reads must be initialized).
            gms = []
            mmean = PS([128, CR], "rows4", 2)
            msqp = PS([128, CR], "rows4", 2)
            nc.tensor.matmul(
                out=mmean[:], lhsT=zeros_row_bf[:], rhs=zeros_row512_bf[:],
                start=True, stop=True, skip_group_check=True,
            )
            nc.tensor.matmul(
                out=msqp[:], lhsT=zeros_row_bf[:], rhs=zeros_row512_bf[:],
                start=True, stop=True, skip_group_check=True,
            )
            for h in range(H):
                vgps = PS([128, CR], "mm", 3)
                nc.tensor.matmul(out=vgps[:], lhsT=Wh("ivw2g", h), rhs=ziv[:], start=True, stop=True)
                p_sb = ck.tile([128, CR], BF16, tag="p_sb", bufs=2)
                nc.vector.tensor_tensor(out=p_sb[:], in0=vgps[:], in1=v0sbs[h][:], op=OP.mult)
                m1ps = PS([128, CR], "mm", 3)
                nc.tensor.matmul(out=m1ps[:], lhsT=W["mw1"][:], rhs=p_sb[:], start=True, stop=False)
                nc.tensor.matmul(out=m1ps[:], lhsT=Wh("WA", h), rhs=cg2, start=False, stop=False)
                nc.tensor.matmul(out=m1ps[:], lhsT=Wh("WB", h), rhs=ziv[:], start=False, stop=True)
                gm = ck.tile([128, CR], BF16, tag=f"gm{h}", bufs=2)
                nc.scalar.activation(out=gm[:], in_=m1ps[:], func=GELU, bias=W["mb1p"][:, h : h + 1])
                gms.append(gm)
                nc.tensor.matmul(
                    out=mmean[32 * h : 32 * h + 1, :], lhsT=inv128_bf[:], rhs=gm[:],
                    start=True, stop=True, tile_position=(0, 32 * h),
                    skip_group_check=True,
                )
                gsq = ck.tile([128, CR], BF16, tag="gsq", bufs=2)
                nc.vector.tensor_tensor(out=gsq[:], in0=gm[:], in1=gm[:], op=OP.mult)
                nc.tensor.matmul(
                    out=msqp[32 * h : 32 * h + 1, :], lhsT=inv128_bf[:], rhs=gsq[:],
                    start=True, stop=True, tile_position=(0, 32 * h),
                    skip_group_check=True,
                )

            carry[0] = (attps, mmean, msqp, gms, qsl, zacc, den_t)

        # epilogue for THIS tile can only be emitted after its chunk-3
        # deferred stage (inside the next tile's first chunk, or right
        # here for the last tile)
        pending_epi[0] = (zacc, den_t, qs)

    deferred_stage(carry[0])
    carry[0] = None
    emit_epilogue(pending_epi[0])
    pending_epi[0] = None

    for p in reversed(_pools):
        p.release()


# ======================= host side =======================


def _host_prep(inputs):
    f = {k: np.asarray(v, np.float32) for k, v in inputs.items()}

    def bf(x):
        return np.ascontiguousarray(np.asarray(x, np.float32)).astype(ml_dtypes.bfloat16)

    def col(x):
        return np.ascontiguousarray(np.asarray(x, np.float32).reshape(-1, 1))

    rff = np.concatenate([FQ * f["rffq"], FV * f["rffv"]], axis=1)  # [2,128]
    # hi/lo split of rff rows for the on-device p@Bs matmul:
    # (r_hi + r_lo)(p_hi + p_lo) ~ r_hi*p_hi + r_hi*p_lo + r_lo*p_hi
    rhi = rff.astype(ml_dtypes.bfloat16).astype(np.float32)
    rlo = rff - rhi
    rff6 = np.zeros((38, 128), np.float32)
    rff6[32] = rhi[0]
    rff6[33] = rhi[0]
    rff6[34] = rlo[0]
    rff6[35] = rhi[1]
    rff6[36] = rhi[1]
    rff6[37] = rlo[1]

    sel4 = np.zeros((4, 128), np.float32)
    for h in range(H):
        sel4[h, 32 * h] = 1.0

    wq_s = f["wq"] * SCALE
    bq_s = f["bq"] * SCALE
    W_qm = f["eq_w2"] @ wq_s
    b_qm = f["eq_b2"] @ wq_s + bq_s
    Mq = np.zeros((128, 512), np.float32)
    w1v = np.zeros((128, 4), np.float32)  # per-head w1v vectors (feature dim)
    w2v = np.zeros((128, 4), np.float32)
    attconst = np.zeros((128, 1), np.float32)
    for h in range(H):
        sl = slice(128 * h, 128 * (h + 1))
        Wq_h = W_qm[:, sl]
        wk_h = f["wk"][:, sl]
        bk_h = f["bk"][sl]
        bq_h = b_qm[sl]
        Mq[:, sl] = Wq_h @ wk_h.T
        w1v[:, h] = wk_h @ bq_h
        w2v[:, h] = Wq_h @ bk_h
        attconst[32 * h, 0] = float(bq_h @ bk_h)

    ivw2f = f["ivls"][:, None] * f["ivw2"]
    ivb2f = f["ivb2"] + f["ivlb"] @ f["ivw2"]
    ivw2g = ivw2f[:, :HD]
    ivw2b = ivw2f[:, HD:]
    # bilinear expansion: m1 = mw1.T (v0*vg) + WA.T cg + WB.T ziv + mb1p
    WA = np.zeros((128, 512), np.float32)
    WB = np.zeros((128, 512), np.float32)
    mb1p = np.zeros((128, H), np.float32)
    for h in range(H):
        sl = slice(128 * h, 128 * (h + 1))
        c1_h = 1.0 + ivb2f[:HD][sl]
        bv_h = f["bv"][sl]
        b2_h = ivb2f[HD:][sl]
        WA[:, sl] = f["wv"][:, sl] @ np.diag(c1_h) @ f["mw1"]
        WB[:, sl] = (ivw2g[:, sl] @ np.diag(bv_h) + ivw2b[:, sl]) @ f["mw1"]
        mb1p[:, h] = f["mb1"] + (bv_h * c1_h + b2_h) @ f["mw1"]

    mw2f = f["mls"][:, None] * f["mw2"]
    mb2f = f["mb2"] + f["mlb"] @ f["mw2"]
    Wmo = np.zeros((128, 512), np.float32)
    for h in range(H):
        wo_h = f["wo"][128 * h : 128 * (h + 1), :]
        Wmo[:, 128 * h : 128 * (h + 1)] = mw2f @ wo_h
    bmo = f["bo"] + sum(mb2f @ f["wo"][128 * h : 128 * (h + 1), :] for h in range(H))

    cw2f = f["cls"][:, None] * f["cw2"]
    cb2f = f["cb2"] + f["clb"] @ f["cw2"]

    weights = {
        "rff": np.ascontiguousarray(rff),
        "rff6": bf(rff6),
        "sel4": bf(sel4),
        "eq_w1": bf(f["eq_w1"]),
        "eq_b1": col(f["eq_b1"]),
        "Mq": bf(Mq),
        "w2v": bf(w2v),
        "attconst": attconst.astype(np.float32),
        "ev_w1": bf(f["ev_w1"]),
        "ev_b1": col(f["ev_b1"]),
        "ev_w2": bf(f["ev_w2"]),
        "ev_b2": col(f["ev_b2"]),
        "ivw1": bf(f["ivw1"]),
        "ivb1": col(f["ivb1"]),
        "ivw2g": bf(ivw2g),
        "wv": bf(f["wv"]),
        "WA": bf(WA),
        "WB": bf(WB),
        "mw1": bf(f["mw1"]),
        "mb1p": np.ascontiguousarray(mb1p),
        "Wmo": bf(Wmo),
        "bmo": col(bmo),
        "cw1": bf(f["cw1"]),
        "cb1": col(f["cb1"]),
        "cw2g": bf(cw2f[:, :DH]),
        "cw2b": bf(cw2f[:, DH:]),
        "cb2g1": col(cb2f[:DH] + 1.0),
        "cb2b": col(cb2f[DH:]),
    }

    x_flat = f["x"].reshape(B * N, CD)
    xh_flat = f["x_h"].reshape(B * N, DH)

    in_maps = []
    for i in range(NCORES):
        b = (i * NQ) // N
        rs = slice(i * NQ, (i + 1) * NQ)
        p_b = f["p"][b]
        c_b = f["c"][b]
        sig_b = f["window_sigma"][b]
        inv2 = 1.0 / (sig_b[:, 0] ** 2)
        phi = p_b.astype(ml_dtypes.bfloat16)
        plo = (p_b - phi.astype(np.float32)).astype(ml_dtypes.bfloat16)
        w1vc = (c_b @ w1v).astype(np.float32)  # [L, 4]
        ctbl = np.zeros((L, 2 * D), ml_dtypes.bfloat16)
        ctbl[:, :D] = bf(c_b)
        ctbl[:, D + 0 : D + 4] = bf(w1vc)
        ctbl[:, D + 32] = phi[:, 0]
        ctbl[:, D + 33] = plo[:, 0]
        ctbl[:, D + 34] = phi[:, 0]
        ctbl[:, D + 35] = phi[:, 1]
        ctbl[:, D + 36] = plo[:, 1]
        ctbl[:, D + 37] = phi[:, 1]
        ctbl[:, D + 64] = (0.5 * inv2).astype(ml_dtypes.bfloat16)
        p2t = np.zeros((3, L), np.float32)
        p2t[0:2] = (2.0 * p_b).T
        p2t[2] = -(p_b**2).sum(1)
        x3 = np.concatenate(
            [x_flat[rs], np.ones((NQ, 1), np.float32)], axis=1
        )
        m = {
            "x": np.ascontiguousarray(x3),
            "xh": np.ascontiguousarray(xh_flat[rs]),
            "ctbl": ctbl,
            "p2t": np.ascontiguousarray(p2t),
        }
        m.update(weights)
        in_maps.append(m)
    return in_maps


_PROGRAM_CACHE = {}


def kernel(**inputs):
    in_maps = _host_prep(inputs)
    if "nc" not in _PROGRAM_CACHE:
        _PROGRAM_CACHE["nc"] = build_program()
    nc = _PROGRAM_CACHE["nc"]

    from concourse.bass_utils import run_bass_kernel_spmd

    res = run_bass_kernel_spmd(nc, in_maps, core_ids=list(range(NCORES)))
    outs = [np.asarray(res.results[i]["out"], np.float32) for i in range(NCORES)]
    return np.concatenate(outs, axis=0).reshape(B, N, DH)


# revision 51
# speedup vs baseline: 1.3140x; 1.0061x over previous
"""EquivariantCrossAttention Trainium2 kernel (8 NeuronCores, SPMD).

kernel(**inputs) takes the FULL unsharded inputs from reference's
setup_inputs() and returns the FULL (B, N, DH) float32 output.

Sharding: flattened query axis (B*N = 4096) split into 8 shards of 512
queries; core i gets queries [512*i, 512*(i+1)) plus its batch's latent
tables. Weights replicated.

Hardcoded problem shapes: B=2 N=2048 L=1024 K=16 CD=2 H=4 DH=128 HD=512.

Algebraic folds done host-side (exact):
  - LayerNorm affines folded into the following Dense weights
  - attention SCALE and eq_w2 folded into wq (W_qm = eq_w2 @ (wq*SCALE))
  - q.k per head via M_h = W_qm_h @ wk_h^T:
      att = g2^T M_h cg + g2.w2v_h + cg.w1v_h + const_h
    The cg.w1v_h term is precomputed per latent on host and gathered.
  - mFFN dense2 and wo merged (W_mo_h = mw2' @ wo_h) and moved after the
    attention sum (softmax weights sum to 1; dense2 affine)
  - mFFN LN normalization folded into attention weights:
      sum_k att*LN(g) = sum_k (att*rstd) g - sum_k att*mean*rstd
  - RFF: t = x@Bs - p@Bs; p@Bs computed on device from hi/lo bf16 split
    of p (3-term product vs hi/lo split of Bs rows);
    sin(2pi t) = Sin(2pi(t - rint t)); cos via +0.25 shift.
  - per-latent gather table row (256 bf16 = 512B):
      [ c (128) | w1vc_h (4) | p0hi p0lo p0hi p1hi p1lo p1hi |
        1/sigma^2 | pad ]
    gathered once per query tile (2048 idxs) with transpose=True.
  - rstd = exp(-0.5*ln(var+eps)) so LN rstds share the ln/exp ACT table
    set with the softmax Exp; mrstd/att_e of chunk c are emitted in
    chunk c+1's ln/exp block (software pipelining) so the scalar engine
    switches table sets only ~2.5x per chunk.

Device structure per core: 4 query tiles x 128 queries; per tile:
scores via PE (|p|^2 folded in via a ones-row) -> top-16 via DVE
max/max_index/match_replace -> one transposed dma_gather -> hoisted
RFF/sin features for all 4 chunks -> 4 chunks of 512 rows (32 queries x
16 neighbors, q-major) through the fused MLP/attention pipeline.
"""

import sys

sys.path.insert(0, "/opt/trn_rl_repo")

import numpy as np
import ml_dtypes

import concourse.bass as bass
import concourse.bacc as bacc
import concourse.mybir as mybir
import concourse.tile as tile
from concourse.masks import make_identity

F32 = mybir.dt.float32
BF16 = mybir.dt.bfloat16
U32 = mybir.dt.uint32
I32 = mybir.dt.int32
I16 = mybir.dt.int16
AF = mybir.ActivationFunctionType
OP = mybir.AluOpType
AX = mybir.AxisListType

B, N, L, K, CD, H, DH, D = 2, 2048, 1024, 16, 2, 4, 128, 128
HD = H * DH
FQ = 2.0
FV = 2.0
SCALE = 1.0 / float(np.sqrt(DH))
NCORES = 8
NQ = (B * N) // NCORES  # queries per core = 512
QT = NQ // 128  # query tiles per core = 4
NCH = 4  # chunks per query tile
CQ = 128 // NCH  # queries per chunk = 32
CR = CQ * K  # rows per chunk = 512
TR = 128 * K  # rows per tile = 2048
GELU = AF.Gelu_apprx_tanh
TWO_PI = 2.0 * np.pi
EPS = 1e-6

WSPECS = [
    ("rff", [CD, 128], F32),
    ("rff6", [38, 128], BF16),
    ("sel4", [4, 128], BF16),
    ("eq_w1", [128, 128], BF16),
    ("eq_b1", [128, 1], F32),
    ("Mq", [128, 512], BF16),
    ("w2v", [128, 4], BF16),
    ("attconst", [128, 1], F32),
    ("ev_w1", [128, 128], BF16),
    ("ev_b1", [128, 1], F32),
    ("ev_w2", [128, 128], BF16),
    ("ev_b2", [128, 1], F32),
    ("ivw1", [128, 128], BF16),
    ("ivb1", [128, 1], F32),
    ("ivw2g", [128, 512], BF16),
    ("wv", [128, 512], BF16),
    ("c1col", [128, 4], F32),
    ("WB", [128, 512], BF16),
    ("mw1", [128, 128], BF16),
    ("mb1p", [128, 4], F32),
    ("Wmo", [128, 512], BF16),
    ("bmo", [128, 1], F32),
    ("cw1", [128, 128], BF16),
    ("cb1", [128, 1], F32),
    ("cw2g", [128, 128], BF16),
    ("cw2b", [128, 128], BF16),
    ("cb2g1", [128, 1], F32),
    ("cb2b", [128, 1], F32),
]


def _bcast_inner(ap, n):
    """[.., Q] AP -> [.., Q, n] with a stride-0 inner dim (free broadcast)."""
    newap = [list(p) for p in ap.ap] + [[0, n]]
    return bass.AP(ap.tensor, ap.offset, newap)


def build_program():
    nc = bacc.Bacc()

    x_d = nc.declare_dram_parameter("x", [NQ, 3], F32, isOutput=False)
    xh_d = nc.declare_dram_parameter("xh", [NQ, DH], F32, isOutput=False)
    ctbl_d = nc.declare_dram_parameter("ctbl", [L, 2 * D], BF16, isOutput=False)
    p2t_d = nc.declare_dram_parameter("p2t", [3, L], F32, isOutput=False)
    w_d = {}
    for name, shape, dt in WSPECS:
        w_d[name] = nc.declare_dram_parameter(name, shape, dt, isOutput=False)
    out_d = nc.declare_dram_parameter("out", [NQ, DH], F32, isOutput=True)

    with tile.TileContext(nc) as tc:
        _emit(nc, tc, x_d, xh_d, ctbl_d, p2t_d, w_d, out_d)
    nc.compile()
    _optimize_act_table_loads(nc)
    return nc


def _optimize_act_table_loads(nc):
    """Remap ln-only/exp-only ACT table-set loads to the combined
    natural_log_exp_and_others set, then drop consecutive reloads of an
    already-resident set. The stock placement picks the first set
    containing each function, which costs a ~1.5us table DMA at every
    Ln<->Exp alternation in the LN-rstd / softmax-exp blocks."""
    from concourse.hw_specs import get_activation_tables

    names = list(get_activation_tables(nc.m.arch).keys())
    try:
        ln_id = names.index("natural_log")
        exp_id = names.index("exp_and_others")
        combo_id = names.index("natural_log_exp_and_others")
    except ValueError:
        return
    for fn in nc.m.functions:
        for blk in fn.blocks:
            dead = []
            cur = {}
            for inst in blk.instructions:
                if isinstance(inst, mybir.InstLoadActFuncSet):
                    if inst.act_func_set_id in (ln_id, exp_id):
                        inst.act_func_set_id = combo_id
                    e = str(inst.engine)
                    si = inst.sync_info
                    clean = si is None or (
                        len(si.on_wait) == 0 and len(si.on_update) == 0
                    )
                    if cur.get(e) == inst.act_func_set_id and clean:
                        dead.append(inst)
                    else:
                        cur[e] = inst.act_func_set_id
            for inst in dead:
                blk.instructions.remove(inst)


def _emit(nc, tc, x_d, xh_d, ctbl_d, p2t_d, w_d, out_d):
    const = tc.alloc_tile_pool(name="const", bufs=1)
    wpool = tc.alloc_tile_pool(name="wpool", bufs=1)
    core = tc.alloc_tile_pool(name="core", bufs=1)
    tl = tc.alloc_tile_pool(name="tl", bufs=1)
    ck = tc.alloc_tile_pool(name="ck", bufs=1)
    psp = tc.alloc_tile_pool(name="psp", bufs=1, space="PSUM")
    _pools = [const, wpool, core, tl, ck, psp]

    _psn = [0]

    def PS(shape, tag, bufs, dtype=F32):
        _psn[0] += 1
        return psp.tile(
            shape, dtype, space="PSUM", tag=tag, bufs=bufs, name=f"ps_{tag}_{_psn[0]}"
        )

    # ---------- constants ----------
    ident = const.tile([128, 128], F32)
    make_identity(nc, ident[:])

    ones_col_bf = const.tile([128, 1], BF16)
    nc.vector.memset(ones_col_bf[:], 1.0)
    inv128_bf = const.tile([128, 1], BF16)
    nc.vector.memset(inv128_bf[:], 1.0 / 128.0)
    ones_row_bf = const.tile([1, 128], BF16)
    nc.vector.memset(ones_row_bf[:], 1.0)
    onesmat_bf = const.tile([128, 128], BF16)
    nc.vector.memset(onesmat_bf[:], 1.0)
    eps_col = const.tile([128, 1], F32)
    nc.vector.memset(eps_col[:], EPS)
    zeros_row_bf = const.tile([1, 128], BF16)
    nc.vector.memset(zeros_row_bf[:], 0.0)
    zeros_row512_bf = const.tile([1, 512], BF16)
    nc.vector.memset(zeros_row512_bf[:], 0.0)

    # ---------- weights ----------
    W = {}
    for name, shape, dt in WSPECS:
        wt = wpool.tile(shape, dt, name=f"w_{name}", tag=f"w_{name}")
        nc.sync.dma_start(out=wt[:], in_=w_d[name][:])
        W[name] = wt

    def Wh(name, h, w=128):
        return W[name][:, h * w : (h + 1) * w]

    # ---------- per-core precompute ----------
    # x arrives with a ones column appended (for the |p|^2 fold in scores)
    x_sb = core.tile([128, QT, 3], F32)
    nc.sync.dma_start(out=x_sb[:], in_=x_d[:].rearrange("(t q) c -> q t c", q=128))
    xsq = core.tile([128, QT], F32)
    xs2 = core.tile([128, QT, CD], F32)
    nc.vector.tensor_tensor(
        out=xs2[:], in0=x_sb[:, :, 0:CD], in1=x_sb[:, :, 0:CD], op=OP.mult
    )
    nc.vector.tensor_reduce(out=xsq[:], in_=xs2[:], axis=AX.X, op=OP.add)

    x_fm = core.tile([3, NQ], F32)
    for t in range(QT):
        tp = PS([3, 128], "mm", 3)
        nc.tensor.transpose(out=tp[:], in_=x_sb[:, t, :], identity=ident[:])
        nc.vector.tensor_copy(out=x_fm[:, 128 * t : 128 * (t + 1)], in_=tp[:])

    p2_fm = core.tile([3, L], F32)
    nc.sync.dma_start(out=p2_fm[:], in_=p2t_d[:])

    # ---------- cFFN on x_h (512 queries at once) ----------
    xh_fm = core.tile([128, NQ], BF16)
    xh_rm = core.tile([128, QT, DH], F32)
    nc.sync.dma_start(out=xh_rm[:], in_=xh_d[:].rearrange("(t q) c -> q t c", q=128))
    for t in range(QT):
        tp = PS([128, 128], "mm", 3)
        nc.tensor.transpose(out=tp[:], in_=xh_rm[:, t, :], identity=ident[:])
        nc.vector.tensor_copy(out=xh_fm[:, 128 * t : 128 * (t + 1)], in_=tp[:])

    c1ps = PS([128, NQ], "mm", 3)
    nc.tensor.matmul(out=c1ps[:], lhsT=W["cw1"][:], rhs=xh_fm[:], start=True, stop=True)
    c1 = core.tile([128, NQ], BF16)
    nc.scalar.activation(out=c1[:], in_=c1ps[:], func=GELU, bias=W["cb1"][:])

    cmean = PS([128, NQ], "rows4", 3)
    nc.tensor.matmul(out=cmean[0:1, :], lhsT=inv128_bf[:], rhs=c1[:], start=True, stop=True)
    c1sq = core.tile([128, NQ], BF16)
    nc.vector.tensor_tensor(out=c1sq[:], in0=c1[:], in1=c1[:], op=OP.mult)
    cmsq = PS([128, NQ], "rows4", 3)
    nc.tensor.matmul(out=cmsq[0:1, :], lhsT=inv128_bf[:], rhs=c1sq[:], start=True, stop=True)

    cm2 = core.tile([1, NQ], F32)
    nc.scalar.square(out=cm2[:], in_=cmean[0:1, :])
    cvar = core.tile([1, NQ], F32)
    nc.vector.tensor_tensor(out=cvar[:], in0=cmsq[0:1, :], in1=cm2[:], op=OP.subtract)
    # rstd = exp(-0.5*ln(var+eps)); stays in the ln/exp table set
    clnv = core.tile([1, NQ], F32)
    nc.scalar.activation(out=clnv[:], in_=cvar[:], func=AF.Ln, bias=eps_col[0:1, :])
    crstd_bf = core.tile([1, NQ], BF16)
    nc.scalar.activation(out=crstd_bf[:], in_=clnv[:], func=AF.Exp, scale=-0.5)
    cmr_bf = core.tile([1, NQ], BF16)
    nc.vector.tensor_tensor(out=cmr_bf[:], in0=cmean[0:1, :], in1=crstd_bf[:], op=OP.mult)
    crbc = PS([128, NQ], "mm", 3)
    nc.tensor.matmul(out=crbc[:], lhsT=ones_row_bf[:], rhs=crstd_bf[:], start=True, stop=True)
    cmbc = PS([128, NQ], "mm", 3)
    nc.tensor.matmul(out=cmbc[:], lhsT=ones_row_bf[:], rhs=cmr_bf[:], start=True, stop=True)
    z1c = core.tile([128, NQ], BF16)
    nc.vector.tensor_tensor(out=z1c[:], in0=c1[:], in1=crbc[:], op=OP.mult)
    zc = core.tile([128, NQ], BF16)
    nc.vector.tensor_tensor(out=zc[:], in0=z1c[:], in1=cmbc[:], op=OP.subtract)

    gp1 = core.tile([128, NQ], BF16)
    modadd = core.tile([128, NQ], BF16)
    gps_ = PS([128, NQ], "mm", 3)
    nc.tensor.matmul(out=gps_[:], lhsT=W["cw2g"][:], rhs=zc[:], start=True, stop=True)
    nc.scalar.activation(out=gp1[:], in_=gps_[:], func=AF.Identity, bias=W["cb2g1"][:])
    btps = PS([128, NQ], "mm", 3)
    nc.tensor.matmul(out=btps[:], lhsT=W["cw2b"][:], rhs=zc[:], start=True, stop=True)
    bt = core.tile([128, NQ], BF16)
    nc.scalar.activation(out=bt[:], in_=btps[:], func=AF.Identity, bias=W["cb2b"][:])
    ma1 = core.tile([128, NQ], BF16)
    nc.vector.tensor_scalar(
        out=ma1[:], in0=gp1[:], scalar1=W["ev_b2"][:], scalar2=None, op0=OP.mult
    )
    nc.vector.tensor_tensor(out=modadd[:], in0=ma1[:], in1=bt[:], op=OP.add)

    # deferred-stage state carried across chunks: (attps, mmean, msqp, gms,
    # qsl, zacc, den_t); tile epilogues are likewise deferred until after
    # the tile's chunk-3 deferred stage has been emitted.
    carry = [None]
    pending_epi = [None]

    def deferred_stage(state):
        """ln/exp block + weighted-sum DVE work for a completed chunk.

        Emitted one chunk later so mrstd/att_e share the ln/exp ACT table
        set with the next chunk's irstd."""
        attps, mmean, msqp, gms, qsl, zacc, den_t = state

        # mm2/mvar/mrstd: only rows {0,32,64,96} carry real stats; other
        # rows hold stale/garbage psum values whose results are never read.
        mm2 = ck.tile([128, CR], F32, tag="mm2")
        nc.scalar.square(out=mm2[:], in_=mmean[:])
        mvar = ck.tile([128, CR], F32, tag="mvar")
        nc.vector.tensor_tensor(out=mvar[:], in0=msqp[:], in1=mm2[:], op=OP.subtract)
        mlnv = ck.tile([128, CR], F32, tag="mlnv")
        nc.scalar.activation(out=mlnv[:], in_=mvar[:], func=AF.Ln, bias=eps_col[:])
        mrstd = ck.tile([128, CR], BF16, tag="mrstd")
        nc.scalar.activation(out=mrstd[:], in_=mlnv[:], func=AF.Exp, scale=-0.5)

        att_e = ck.tile([128, CR], BF16, tag="att_e")
        nc.scalar.activation(out=att_e[:], in_=attps[:], func=AF.Exp, bias=W["attconst"][:])
        nc.vector.tensor_reduce(
            out=den_t[:, qsl], in_=att_e[:].rearrange("p (a b) -> p a b", a=CQ),
            axis=AX.X, op=OP.add,
        )

        a2 = ck.tile([128, CR], BF16, tag="a2")
        nc.vector.tensor_tensor(out=a2[:], in0=att_e[:], in1=mrstd[:], op=OP.mult)
        # a3 = att_e * mean * rstd = a2 * mean  (psum operand)
        a3 = ck.tile([128, CR], BF16, tag="a3")
        nc.vector.tensor_tensor(out=a3[:], in0=a2[:], in1=mmean[:], op=OP.mult)
        s3bf = ck.tile([128, CQ], BF16, tag="s3bf")
        with nc.allow_low_precision(reason="16-term bf16 row sum; folded into bf16 matmul anyway"):
            nc.vector.tensor_reduce(
                out=s3bf[:], in_=a3[:].rearrange("p (a b) -> p a b", a=CQ), axis=AX.X,
                op=OP.add,
            )

        for h in range(H):
            a2bc = PS([128, CR], "mm", 3)
            nc.tensor.matmul(
                out=a2bc[:], lhsT=onesmat_bf[32 * h : 32 * h + 1, :],
                rhs=a2[32 * h : 32 * h + 1, :], start=True, stop=True,
                tile_position=(32 * h, 0),
            )
            zp = ck.tile([128, CR], BF16, tag="zp", bufs=2)
            nc.vector.tensor_tensor(out=zp[:], in0=gms[h][:], in1=a2bc[:], op=OP.mult)
            nc.vector.tensor_reduce(
                out=zacc[h][:, qsl], in_=zp[:].rearrange("p (a b) -> p a b", a=CQ),
                axis=AX.X, op=OP.add,
            )
            s3bc = PS([128, CQ], "mm", 3)
            nc.tensor.matmul(
                out=s3bc[:], lhsT=onesmat_bf[32 * h : 32 * h + 1, :],
                rhs=s3bf[32 * h : 32 * h + 1, :], start=True, stop=True,
                tile_position=(32 * h, 0),
            )
            nc.vector.tensor_tensor(
                out=zacc[h][:, qsl], in0=zacc[h][:, qsl], in1=s3bc[:], op=OP.subtract
            )

    def emit_epilogue(state):
        """Softmax denominator fold + output matmul/transpose/store for a
        finished tile (emitted after the tile's chunk-3 deferred stage)."""
        zacc, den_t, qs = state
        rden_t = tl.tile([128, 128], F32, tag="rden_t", bufs=2)
        nc.vector.reciprocal(out=rden_t[:], in_=den_t[:])
        rdbf = tl.tile([128, 128], BF16, tag="rdbf", bufs=2)
        nc.vector.tensor_copy(out=rdbf[:], in_=rden_t[:])
        for h in range(H):
            rdbc = PS([128, 128], "mm", 3)
            nc.tensor.matmul(
                out=rdbc[:], lhsT=onesmat_bf[32 * h : 32 * h + 1, :],
                rhs=rdbf[32 * h : 32 * h + 1, :], start=True, stop=True,
                tile_position=(32 * h, 0),
            )
            nc.vector.tensor_tensor(out=zacc[h][:], in0=zacc[h][:], in1=rdbc[:], op=OP.mult)

        outps = PS([128, 128], "mm", 3)
        for h in range(H):
            zbf = tl.tile([128, 128], BF16, tag="zbf")
            nc.vector.tensor_copy(out=zbf[:], in_=zacc[h][:])
            nc.tensor.matmul(
                out=outps[:], lhsT=Wh("Wmo", h), rhs=zbf[:], start=(h == 0), stop=(h == H - 1)
            )
        outsb = tl.tile([128, 128], F32, tag="outsb")
        nc.scalar.activation(out=outsb[:], in_=outps[:], func=AF.Identity, bias=W["bmo"][:])
        trp = PS([128, 128], "mm", 3)
        nc.tensor.transpose(out=trp[:], in_=outsb[:], identity=ident[:])
        outrm = tl.tile([128, 128], F32, tag="outrm")
        nc.vector.tensor_copy(out=outrm[:], in_=trp[:])
        nc.sync.dma_start(out=out_d[qs, :], in_=outrm[:])

    # ---------- prologue: scores -> top-16 -> gathers for ALL tiles ----
    # Emitting every tile's gather chain up front queues all 16 gathers on
    # the gpsimd engine from the start, so no tile boundary ever stalls
    # waiting for its gather (the chunk pipeline consumes gather #k around
    # t~60k us while it completes around t~11k us).
    cfm_all = []
    gw_all = []
    for t in range(QT):
        qs = slice(128 * t, 128 * (t + 1))

        # scores (two 512-wide halves; |p|^2 folded via ones row) + top-16
        scores = tl.tile([128, L], F32, tag="scores")
        for s in range(2):
            sl = slice(512 * s, 512 * (s + 1))
            scps = PS([128, 512], "mm", 3)
            nc.tensor.matmul(out=scps[:], lhsT=x_fm[:, qs], rhs=p2_fm[:, sl], start=True, stop=True)
            nc.scalar.copy(out=scores[:, sl], in_=scps[:])
        vals = tl.tile([128, K], F32, tag="vals", bufs=2)
        idxs = tl.tile([128, K], U32, tag="idxs", bufs=2)
        scr2 = tl.tile([128, L], F32, tag="scr2")
        nc.vector.max(out=vals[:, 0:8], in_=scores[:])
        nc.vector.max_index(out=idxs[:, 0:8], in_max=vals[:, 0:8], in_values=scores[:])
        nc.vector.match_replace(
            out=scr2[:], in_to_replace=vals[:, 0:8], in_values=scores[:], imm_value=-1e30
        )
        nc.vector.max(out=vals[:, 8:16], in_=scr2[:])
        nc.vector.max_index(out=idxs[:, 8:16], in_max=vals[:, 8:16], in_values=scr2[:])

        # index prep for dma_gather: idx16[k, q] (int16) replicated across
        # the 8 gpsimd cores' 16-partition blocks
        idxf = tl.tile([128, K], F32, tag="idxf", bufs=2)
        nc.vector.tensor_copy(out=idxf[:], in_=idxs[:])
        idxt_ps = PS([K, 128], "mm", 3)
        nc.tensor.transpose(out=idxt_ps[:], in_=idxf[:], identity=ident[:])
        idx16 = tl.tile([128, 128], I16, tag="idx16", bufs=QT)
        nc.vector.tensor_copy(out=idx16[0:16, :], in_=idxt_ps[:])
        for b in range(1, 8):
            nc.sync.dma_start(out=idx16[16 * b : 16 * b + 16, :], in_=idx16[0:16, :])

        # transposed gathers (512B rows): cfm [128, 2, 512] per chunk:
        # block 0 = c features; block 1 rows 0-3 = w1v.c per head, rows
        # 32-37 = p hi/lo splits, row 64 = 0.5/sigma^2 (row choices keep
        # every consumer's partition bases legal/aligned)
        cfm_cs = []
        for gc in range(NCH):
            cfm_c = tl.tile([128, 2, CR], BF16, tag=f"cfm{gc}", bufs=3,
                            name=f"cfm{gc}_{t}")
            nc.gpsimd.dma_gather(
                out_ap=cfm_c[:], in_ap=ctbl_d[:],
                idxs_ap=idx16[:, 32 * gc : 32 * gc + 32], num_idxs=CR,
                num_idxs_reg=CR, elem_size=2 * D, transpose=True,
            )
            cfm_cs.append(cfm_c)
        cfm_all.append(cfm_cs)

        # -d^2 (q-major, [128, K]); flattened to a row in the main loop
        negd2 = tl.tile([128, K], BF16, tag="negd2", bufs=QT)
        nc.vector.tensor_scalar(
            out=negd2[:], in0=vals[:], scalar1=xsq[:, t : t + 1], scalar2=None,
            op0=OP.subtract,
        )
        gw_all.append(negd2)

    # ---------- per query tile ----------
    for t in range(QT):
        qs = slice(128 * t, 128 * (t + 1))
        cfm_cs = cfm_all[t]
        negd2 = gw_all[t]

        # -d^2 row DMA-flattened INTO partition 64 so the gw multiply is
        # partition-base-aligned with the gathered 0.5/sigma^2 row.
        # Emitted here, not in the prologue, so the in-order DVE queue
        # never blocks on a still-running gather.
        negd2_row = tl.tile([65, TR], BF16, tag="negd2_row", bufs=2)
        nc.sync.dma_start(out=negd2_row[64:65, :], in_=negd2[:])
        gw_t = tl.tile([65, TR], BF16, tag="gw_t", bufs=2)
        for gc in range(NCH):
            nc.vector.tensor_tensor(
                out=gw_t[64:65, CR * gc : CR * (gc + 1)],
                in0=negd2_row[64:65, CR * gc : CR * (gc + 1)],
                in1=cfm_cs[gc][64:65, 1, :], op=OP.mult,
            )

        # t_x = x @ Bs for this tile [128, 128]
        txps = PS([128, 128], "mm", 3)
        nc.tensor.matmul(out=txps[:], lhsT=W["rff"][:], rhs=x_fm[0:2, qs], start=True, stop=True)
        t_x = tl.tile([128, 128], F32, tag="t_x", bufs=2)
        nc.vector.tensor_copy(out=t_x[:], in_=txps[:])

        # ---- hoisted RFF features for all 4 chunks; the 16 Sin calls land
        # contiguously in the scalar queue (one trig table load per tile)
        fqs, fvs = [], []
        for c in range(NCH):
            q0 = CQ * c
            qsl = slice(q0, q0 + CQ)
            cs = slice(CR * c, CR * (c + 1))
            # p@Bs from hi/lo split: 6-row bf16 matmul (table rows 32-37)
            pbs = PS([128, CR], "mm", 3)
            nc.tensor.matmul(
                out=pbs[:], lhsT=W["rff6"][32:38, :], rhs=cfm_cs[c][32:38, 1, :],
                start=True, stop=True,
            )
            # frac chain; paired ops share a tag (lifetimes don't overlap)
            tfull = ck.tile([128, CQ, K], F32, tag="tfrac", bufs=2)
            nc.vector.tensor_tensor(
                out=tfull[:], in0=_bcast_inner(t_x[:, qsl], K),
                in1=pbs[:].rearrange("p (a b) -> p a b", a=CQ), op=OP.subtract,
            )
            ti = ck.tile([128, CQ, K], I32, tag="ifrac", bufs=2)
            nc.vector.tensor_copy(out=ti[:], in_=tfull[:])
            fs = ck.tile([128, CQ, K], F32, tag="sfrac", bufs=2)
            nc.vector.tensor_tensor(out=fs[:], in0=tfull[:], in1=ti[:], op=OP.subtract)
            dc0 = ck.tile([128, CQ, K], F32, tag="tfrac", bufs=2)
            nc.vector.tensor_scalar(
                out=dc0[:], in0=tfull[:], scalar1=0.25, scalar2=None, op0=OP.add
            )
            ui = ck.tile([128, CQ, K], I32, tag="ifrac", bufs=2)
            nc.vector.tensor_copy(out=ui[:], in_=dc0[:])
            dc = ck.tile([128, CQ, K], F32, tag="sfrac", bufs=2)
            nc.vector.tensor_tensor(out=dc[:], in0=dc0[:], in1=ui[:], op=OP.subtract)
            fq = tl.tile([128, CR], BF16, tag=f"fq{c}", bufs=2, name=f"fq{c}_{t}")
            fv = tl.tile([128, CR], BF16, tag=f"fv{c}", bufs=2, name=f"fv{c}_{t}")
            fs2 = fs[:].rearrange("p a b -> p (a b)")
            dc2 = dc[:].rearrange("p a b -> p (a b)")
            nc.scalar.activation(out=fq[0:64, :], in_=fs2[0:64, :], func=AF.Sin, scale=TWO_PI)
            nc.scalar.activation(out=fq[64:128, :], in_=dc2[0:64, :], func=AF.Sin, scale=TWO_PI)
            nc.scalar.activation(out=fv[0:64, :], in_=fs2[64:128, :], func=AF.Sin, scale=TWO_PI)
            nc.scalar.activation(out=fv[64:128, :], in_=dc2[64:128, :], func=AF.Sin, scale=TWO_PI)
            fqs.append(fq)
            fvs.append(fv)

        zacc = [
            tl.tile([128, 128], F32, tag=f"zacc{h}", name=f"zacc{h}_{t}", bufs=2)
            for h in range(H)
        ]
        den_t = tl.tile([128, 128], F32, tag="den_t", bufs=2, name=f"den_{t}")

        # ---------- chunks ----------
        for c in range(NCH):
            q0 = CQ * c
            qsl = slice(q0, q0 + CQ)
            gq = slice(128 * t + q0, 128 * t + q0 + CQ)
            cs = slice(CR * c, CR * (c + 1))
            cg2 = cfm_cs[c][:, 0, :]

            # q path
            g2ps = PS([128, CR], "mm", 3)
            nc.tensor.matmul(out=g2ps[:], lhsT=W["eq_w1"][:], rhs=fqs[c][:], start=True, stop=True)
            g2 = ck.tile([128, CR], BF16, tag="g2", bufs=2)
            nc.scalar.activation(out=g2[:], in_=g2ps[:], func=GELU, bias=W["eq_b1"][:])

            # attention logits in one psum tile, rows {0,32,64,96}.
            # seed = 0.5*gw broadcast to all rows (start=True).
            attps = PS([128, CR], "attps", 2)
            nc.tensor.matmul(
                out=attps[:], lhsT=onesmat_bf[64:65, :],
                rhs=gw_t[64:65, cs], start=True, stop=False,
                skip_group_check=True,
            )
            # w2v^T g2 for all heads [4, CR] + gathered w1v.c, scattered to
            # rows {32h} by one selector matmul
            w24 = PS([128, CR], "mm", 3)
            nc.tensor.matmul(out=w24[0:4, :], lhsT=W["w2v"][:], rhs=g2[:], start=True, stop=True)
            wsum = ck.tile([4, CR], BF16, tag="wsum", bufs=2)
            nc.vector.tensor_tensor(
                out=wsum[:], in0=w24[0:4, :], in1=cfm_cs[c][0:4, 1, :], op=OP.add
            )
            nc.tensor.matmul(
                out=attps[:], lhsT=W["sel4"][:], rhs=wsum[:],
                start=False, stop=False, skip_group_check=True,
            )
            for h in range(H):
                ups = PS([128, CR], "mm", 3)
                nc.tensor.matmul(out=ups[:], lhsT=Wh("Mq", h), rhs=g2[:], start=True, stop=True)
                qkp = ck.tile([128, CR], BF16, tag="qkp", bufs=2)
                nc.vector.tensor_tensor(out=qkp[:], in0=ups[:], in1=cg2, op=OP.mult)
                nc.tensor.matmul(
                    out=attps[32 * h : 32 * h + 1, :], lhsT=ones_col_bf[:], rhs=qkp[:],
                    start=False, stop=(h == H - 1), tile_position=(0, 32 * h),
                    skip_group_check=True,
                )

            # v path
            ev1ps = PS([128, CR], "mm", 3)
            nc.tensor.matmul(out=ev1ps[:], lhsT=W["ev_w1"][:], rhs=fvs[c][:], start=True, stop=True)
            ev1 = ck.tile([128, CR], BF16, tag="ev1", bufs=2)
            nc.scalar.activation(out=ev1[:], in_=ev1ps[:], func=GELU, bias=W["ev_b1"][:])
            ev2ps = PS([128, CR], "mm", 3)
            nc.tensor.matmul(out=ev2ps[:], lhsT=W["ev_w2"][:], rhs=ev1[:], start=True, stop=True)
            mv = ck.tile([128, CQ, K], BF16, tag="mv", bufs=2)
            nc.vector.tensor_tensor(
                out=mv[:], in0=ev2ps[:].rearrange("p (a b) -> p a b", a=CQ),
                in1=_bcast_inner(gp1[:, gq], K), op=OP.mult,
            )
            ivin = ck.tile([128, CQ, K], BF16, tag="ivin", bufs=2)
            nc.vector.tensor_tensor(
                out=ivin[:], in0=mv[:], in1=_bcast_inner(modadd[:, gq], K), op=OP.add
            )
            iv1ps = PS([128, CR], "mm", 3)
            nc.tensor.matmul(
                out=iv1ps[:], lhsT=W["ivw1"][:],
                rhs=ivin[:].rearrange("p a b -> p (a b)"), start=True, stop=True,
            )
            iv1 = ck.tile([128, CR], BF16, tag="iv1", bufs=2)
            nc.scalar.activation(out=iv1[:], in_=iv1ps[:], func=GELU, bias=W["ivb1"][:])

            # iv LN (rstd via ln/exp; the ln/exp block below also carries the
            # previous chunk's deferred mrstd/att_e work). Mean at psum row
            # 0, mean-of-squares at row 32 of ONE tile (fewer psum allocs);
            # the var math runs at partition base 32 (legal base), with ACT
            # crossing partitions where needed.
            ivstats = PS([128, CR], "rows4", 3)
            nc.tensor.matmul(
                out=ivstats[0:1, :], lhsT=inv128_bf[:], rhs=iv1[:],
                start=True, stop=True, skip_group_check=True,
            )
            ivsq = ck.tile([128, CR], BF16, tag="ivsq", bufs=2)
            nc.vector.tensor_tensor(out=ivsq[:], in0=iv1[:], in1=iv1[:], op=OP.mult)
            nc.tensor.matmul(
                out=ivstats[32:33, :], lhsT=inv128_bf[:], rhs=ivsq[:],
                start=True, stop=True, tile_position=(0, 32),
                skip_group_check=True,
            )
            im2 = ck.tile([33, CR], F32, tag="im2")
            nc.scalar.square(out=im2[32:33, :], in_=ivstats[0:1, :])
            ivar = ck.tile([33, CR], F32, tag="ivar")
            nc.vector.tensor_tensor(
                out=ivar[32:33, :], in0=ivstats[32:33, :], in1=im2[32:33, :],
                op=OP.subtract,
            )

            ilnv = ck.tile([1, CR], F32, tag="ilnv")
            nc.scalar.activation(out=ilnv[:], in_=ivar[32:33, :], func=AF.Ln, bias=eps_col[0:1, :])
            irstd_bf = ck.tile([1, CR], BF16, tag="irstd_bf")
            nc.scalar.activation(out=irstd_bf[:], in_=ilnv[:], func=AF.Exp, scale=-0.5)

            # deferred ln/exp + weighted-sum work of the previous chunk,
            # then (entering a new tile) the previous tile's epilogue
            if carry[0] is not None:
                deferred_stage(carry[0])
                carry[0] = None
            if c == 0 and pending_epi[0] is not None:
                emit_epilogue(pending_epi[0])
                pending_epi[0] = None

            imr_bf = ck.tile([1, CR], BF16, tag="imr_bf")
            nc.vector.tensor_tensor(
                out=imr_bf[:], in0=ivstats[0:1, :], in1=irstd_bf[:], op=OP.mult
            )
            irbc = PS([128, CR], "mm", 3)
            nc.tensor.matmul(out=irbc[:], lhsT=ones_row_bf[:], rhs=irstd_bf[:], start=True, stop=True)
            imbc = PS([128, CR], "mm", 3)
            nc.tensor.matmul(out=imbc[:], lhsT=ones_row_bf[:], rhs=imr_bf[:], start=True, stop=True)
            z1 = ck.tile([128, CR], BF16, tag="z1", bufs=2)
            nc.vector.tensor_tensor(out=z1[:], in0=iv1[:], in1=irbc[:], op=OP.mult)
            ziv = ck.tile([128, CR], BF16, tag="ziv", bufs=2)
            nc.vector.tensor_tensor(out=ziv[:], in0=z1[:], in1=imbc[:], op=OP.subtract)

            # per-head v1 -> gm ; m-LN stats to psum rows {32h}.
            # DVE memset seeds the rows the per-head matmuls don't write
            # (their values are never used, but reads must be initialized).
            # The WA (conditioning-gate constant) term is folded into the
            # vg->bf16 copy as a per-feature bias: m1 = mw1^T((g^+c1).v0)
            # + WB^T ziv, saving one PE stream per head.
            gms = []
            mmean = PS([128, CR], "rows4", 3)
            msqp = PS([128, CR], "rows4", 3)
            nc.vector.memset(mmean[:], 0.0)
            nc.vector.memset(msqp[:], 0.0)
            for h in range(H):
                vgps = PS([128, CR], "mm", 3)
                nc.tensor.matmul(out=vgps[:], lhsT=Wh("ivw2g", h), rhs=ziv[:], start=True, stop=True)
                gc_sb = ck.tile([128, CR], BF16, tag="gc_sb", bufs=2)
                nc.scalar.activation(
                    out=gc_sb[:], in_=vgps[:], func=AF.Identity,
                    bias=W["c1col"][:, h : h + 1],
                )
                v0ps = PS([128, CR], "mm", 3)
                nc.tensor.matmul(out=v0ps[:], lhsT=Wh("wv", h), rhs=cg2, start=True, stop=True)
                p_sb = ck.tile([128, CR], BF16, tag="p_sb", bufs=2)
                nc.vector.tensor_tensor(out=p_sb[:], in0=v0ps[:], in1=gc_sb[:], op=OP.mult)
                m1ps = PS([128, CR], "mm", 3)
                nc.tensor.matmul(out=m1ps[:], lhsT=W["mw1"][:], rhs=p_sb[:], start=True, stop=False)
                nc.tensor.matmul(out=m1ps[:], lhsT=Wh("WB", h), rhs=ziv[:], start=False, stop=True)
                gm = ck.tile([128, CR], BF16, tag=f"gm{h}", bufs=2)
                nc.scalar.activation(out=gm[:], in_=m1ps[:], func=GELU, bias=W["mb1p"][:, h : h + 1])
                gms.append(gm)
                nc.tensor.matmul(
                    out=mmean[32 * h : 32 * h + 1, :], lhsT=inv128_bf[:], rhs=gm[:],
                    start=True, stop=True, tile_position=(0, 32 * h),
                    skip_group_check=True,
                )
                gsq = ck.tile([128, CR], BF16, tag="gsq", bufs=2)
                nc.vector.tensor_tensor(out=gsq[:], in0=gm[:], in1=gm[:], op=OP.mult)
                nc.tensor.matmul(
                    out=msqp[32 * h : 32 * h + 1, :], lhsT=inv128_bf[:], rhs=gsq[:],
                    start=True, stop=True, tile_position=(0, 32 * h),
                    skip_group_check=True,
                )

            carry[0] = (attps, mmean, msqp, gms, qsl, zacc, den_t)

        # epilogue for THIS tile can only be emitted after its chunk-3
        # deferred stage (inside the next tile's first chunk, or right
        # here for the last tile)
        pending_epi[0] = (zacc, den_t, qs)

    deferred_stage(carry[0])
    carry[0] = None
    emit_epilogue(pending_epi[0])
    pending_epi[0] = None

    for p in reversed(_pools):
        p.release()


# ======================= host side =======================


def _host_prep(inputs):
    f = {k: np.asarray(v, np.float32) for k, v in inputs.items()}

    def bf(x):
        return np.ascontiguousarray(np.asarray(x, np.float32)).astype(ml_dtypes.bfloat16)

    def col(x):
        return np.ascontiguousarray(np.asarray(x, np.float32).reshape(-1, 1))

    rff = np.concatenate([FQ * f["rffq"], FV * f["rffv"]], axis=1)  # [2,128]
    # hi/lo split of rff rows for the on-device p@Bs matmul:
    # (r_hi + r_lo)(p_hi + p_lo) ~ r_hi*p_hi + r_hi*p_lo + r_lo*p_hi
    rhi = rff.astype(ml_dtypes.bfloat16).astype(np.float32)
    rlo = rff - rhi
    rff6 = np.zeros((38, 128), np.float32)
    rff6[32] = rhi[0]
    rff6[33] = rhi[0]
    rff6[34] = rlo[0]
    rff6[35] = rhi[1]
    rff6[36] = rhi[1]
    rff6[37] = rlo[1]

    sel4 = np.zeros((4, 128), np.float32)
    for h in range(H):
        sel4[h, 32 * h] = 1.0

    wq_s = f["wq"] * SCALE
    bq_s = f["bq"] * SCALE
    W_qm = f["eq_w2"] @ wq_s
    b_qm = f["eq_b2"] @ wq_s + bq_s
    Mq = np.zeros((128, 512), np.float32)
    w1v = np.zeros((128, 4), np.float32)  # per-head w1v vectors (feature dim)
    w2v = np.zeros((128, 4), np.float32)
    attconst = np.zeros((128, 1), np.float32)
    for h in range(H):
        sl = slice(128 * h, 128 * (h + 1))
        Wq_h = W_qm[:, sl]
        wk_h = f["wk"][:, sl]
        bk_h = f["bk"][sl]
        bq_h = b_qm[sl]
        Mq[:, sl] = Wq_h @ wk_h.T
        w1v[:, h] = wk_h @ bq_h
        w2v[:, h] = Wq_h @ bk_h
        attconst[32 * h, 0] = float(bq_h @ bk_h)

    ivw2f = f["ivls"][:, None] * f["ivw2"]
    ivb2f = f["ivb2"] + f["ivlb"] @ f["ivw2"]
    ivw2g = ivw2f[:, :HD]
    ivw2b = ivw2f[:, HD:]
    # bilinear expansion: m1 = mw1.T ((vg+c1)*v0) + WB.T ziv + mb1p
    # (the WA = wv diag(c1) mw1 term folds into the vg copy's c1 bias)
    c1col = np.zeros((128, H), np.float32)
    WB = np.zeros((128, 512), np.float32)
    mb1p = np.zeros((128, H), np.float32)
    for h in range(H):
        sl = slice(128 * h, 128 * (h + 1))
        c1_h = 1.0 + ivb2f[:HD][sl]
        bv_h = f["bv"][sl]
        b2_h = ivb2f[HD:][sl]
        c1col[:, h] = c1_h
        WB[:, sl] = (ivw2g[:, sl] @ np.diag(bv_h) + ivw2b[:, sl]) @ f["mw1"]
        mb1p[:, h] = f["mb1"] + (bv_h * c1_h + b2_h) @ f["mw1"]

    mw2f = f["mls"][:, None] * f["mw2"]
    mb2f = f["mb2"] + f["mlb"] @ f["mw2"]
    Wmo = np.zeros((128, 512), np.float32)
    for h in range(H):
        wo_h = f["wo"][128 * h : 128 * (h + 1), :]
        Wmo[:, 128 * h : 128 * (h + 1)] = mw2f @ wo_h
    bmo = f["bo"] + sum(mb2f @ f["wo"][128 * h : 128 * (h + 1), :] for h in range(H))

    cw2f = f["cls"][:, None] * f["cw2"]
    cb2f = f["cb2"] + f["clb"] @ f["cw2"]

    weights = {
        "rff": np.ascontiguousarray(rff),
        "rff6": bf(rff6),
        "sel4": bf(sel4),
        "eq_w1": bf(f["eq_w1"]),
        "eq_b1": col(f["eq_b1"]),
        "Mq": bf(Mq),
        "w2v": bf(w2v),
        "attconst": attconst.astype(np.float32),
        "ev_w1": bf(f["ev_w1"]),
        "ev_b1": col(f["ev_b1"]),
        "ev_w2": bf(f["ev_w2"]),
        "ev_b2": col(f["ev_b2"]),
        "ivw1": bf(f["ivw1"]),
        "ivb1": col(f["ivb1"]),
        "ivw2g": bf(ivw2g),
        "wv": bf(f["wv"]),
        "c1col": np.ascontiguousarray(c1col),
        "WB": bf(WB),
        "mw1": bf(f["mw1"]),
        "mb1p": np.ascontiguousarray(mb1p),
        "Wmo": bf(Wmo),
        "bmo": col(bmo),
        "cw1": bf(f["cw1"]),
        "cb1": col(f["cb1"]),
        "cw2g": bf(cw2f[:, :DH]),
        "cw2b": bf(cw2f[:, DH:]),
        "cb2g1": col(cb2f[:DH] + 1.0),
        "cb2b": col(cb2f[DH:]),
    }

    x_flat = f["x"].reshape(B * N, CD)
    xh_flat = f["x_h"].reshape(B * N, DH)

    in_maps = []
    for i in range(NCORES):
        b = (i * NQ) // N
        rs = slice(i * NQ, (i + 1) * NQ)
        p_b = f["p"][b]
        c_b = f["c"][b]
        sig_b = f["window_sigma"][b]
        inv2 = 1.0 / (sig_b[:, 0] ** 2)
        phi = p_b.astype(ml_dtypes.bfloat16)
        plo = (p_b - phi.astype(np.float32)).astype(ml_dtypes.bfloat16)
        w1vc = (c_b @ w1v).astype(np.float32)  # [L, 4]
        ctbl = np.zeros((L, 2 * D), ml_dtypes.bfloat16)
        ctbl[:, :D] = bf(c_b)
        ctbl[:, D + 0 : D + 4] = bf(w1vc)
        ctbl[:, D + 32] = phi[:, 0]
        ctbl[:, D + 33] = plo[:, 0]
        ctbl[:, D + 34] = phi[:, 0]
        ctbl[:, D + 35] = phi[:, 1]
        ctbl[:, D + 36] = plo[:, 1]
        ctbl[:, D + 37] = phi[:, 1]
        ctbl[:, D + 64] = (0.5 * inv2).astype(ml_dtypes.bfloat16)
        p2t = np.zeros((3, L), np.float32)
        p2t[0:2] = (2.0 * p_b).T
        p2t[2] = -(p_b**2).sum(1)
        x3 = np.concatenate(
            [x_flat[rs], np.ones((NQ, 1), np.float32)], axis=1
        )
        m = {
            "x": np.ascontiguousarray(x3),
            "xh": np.ascontiguousarray(xh_flat[rs]),
            "ctbl": ctbl,
            "p2t": np.ascontiguousarray(p2t),
        }
        m.update(weights)
        in_maps.append(m)
    return in_maps


_PROGRAM_CACHE = {}


def kernel(**inputs):
    in_maps = _host_prep(inputs)
    if "nc" not in _PROGRAM_CACHE:
        _PROGRAM_CACHE["nc"] = build_program()
    nc = _PROGRAM_CACHE["nc"]

    from concourse.bass_utils import run_bass_kernel_spmd

    res = run_bass_kernel_spmd(nc, in_maps, core_ids=list(range(NCORES)))
    outs = [np.asarray(res.results[i]["out"], np.float32) for i in range(NCORES)]
    return np.concatenate(outs, axis=0).reshape(B, N, DH)


# revision 57
# speedup vs baseline: 1.3171x; 1.0024x over previous
"""EquivariantCrossAttention Trainium2 kernel (8 NeuronCores, SPMD).

kernel(**inputs) takes the FULL unsharded inputs from reference's
setup_inputs() and returns the FULL (B, N, DH) float32 output.

Sharding: flattened query axis (B*N = 4096) split into 8 shards of 512
queries; core i gets queries [512*i, 512*(i+1)) plus its batch's latent
tables. Weights replicated.

Hardcoded problem shapes: B=2 N=2048 L=1024 K=16 CD=2 H=4 DH=128 HD=512.

Algebraic folds done host-side (exact):
  - LayerNorm affines folded into the following Dense weights
  - attention SCALE and eq_w2 folded into wq (W_qm = eq_w2 @ (wq*SCALE))
  - q.k per head via M_h = W_qm_h @ wk_h^T:
      att = g2^T M_h cg + g2.w2v_h + cg.w1v_h + const_h
    The cg.w1v_h term is precomputed per latent on host and gathered.
  - mFFN dense2 and wo merged (W_mo_h = mw2' @ wo_h) and moved after the
    attention sum (softmax weights sum to 1; dense2 affine)
  - mFFN LN normalization folded into attention weights:
      sum_k att*LN(g) = sum_k (att*rstd) g - sum_k att*mean*rstd
  - RFF: t = x@Bs - p@Bs; p@Bs computed on device from hi/lo bf16 split
    of p (3-term product vs hi/lo split of Bs rows);
    sin(2pi t) = Sin(2pi(t - rint t)); cos via +0.25 shift.
  - per-latent gather table row (256 bf16 = 512B):
      [ c (128) | w1vc_h (4) | p0hi p0lo p0hi p1hi p1lo p1hi |
        1/sigma^2 | pad ]
    gathered once per query tile (2048 idxs) with transpose=True.
  - rstd = exp(-0.5*ln(var+eps)) so LN rstds share the ln/exp ACT table
    set with the softmax Exp; mrstd/att_e of chunk c are emitted in
    chunk c+1's ln/exp block (software pipelining) so the scalar engine
    switches table sets only ~2.5x per chunk.

Device structure per core: 4 query tiles x 128 queries; per tile:
scores via PE (|p|^2 folded in via a ones-row) -> top-16 via DVE
max/max_index/match_replace -> one transposed dma_gather -> hoisted
RFF/sin features for all 4 chunks -> 4 chunks of 512 rows (32 queries x
16 neighbors, q-major) through the fused MLP/attention pipeline.
"""

import sys

sys.path.insert(0, "/opt/trn_rl_repo")

import numpy as np
import ml_dtypes

import concourse.bass as bass
import concourse.bacc as bacc
import concourse.mybir as mybir
import concourse.tile as tile
from concourse.masks import make_identity

F32 = mybir.dt.float32
BF16 = mybir.dt.bfloat16
U32 = mybir.dt.uint32
I32 = mybir.dt.int32
I16 = mybir.dt.int16
AF = mybir.ActivationFunctionType
OP = mybir.AluOpType
AX = mybir.AxisListType

B, N, L, K, CD, H, DH, D = 2, 2048, 1024, 16, 2, 4, 128, 128
HD = H * DH
FQ = 2.0
FV = 2.0
SCALE = 1.0 / float(np.sqrt(DH))
NCORES = 8
NQ = (B * N) // NCORES  # queries per core = 512
QT = NQ // 128  # query tiles per core = 4
NCH = 4  # chunks per query tile
CQ = 128 // NCH  # queries per chunk = 32
CR = CQ * K  # rows per chunk = 512
TR = 128 * K  # rows per tile = 2048
GELU = AF.Gelu_apprx_tanh
TWO_PI = 2.0 * np.pi
EPS = 1e-6

WSPECS = [
    ("rff", [CD, 128], F32),
    ("rff6", [38, 128], BF16),
    ("sel4", [4, 128], BF16),
    ("eq_w1", [128, 128], BF16),
    ("eq_b1", [128, 1], F32),
    ("Mq", [128, 512], BF16),
    ("w2v", [128, 4], BF16),
    ("attconst", [128, 1], F32),
    ("ev_w1", [128, 128], BF16),
    ("ev_b1", [128, 1], F32),
    ("ev_w2", [128, 128], BF16),
    ("ev_b2", [128, 1], F32),
    ("ivw1", [128, 128], BF16),
    ("ivb1", [128, 1], F32),
    ("ivw2g", [128, 512], BF16),
    ("wv", [128, 512], BF16),
    ("c1col", [128, 4], F32),
    ("WB", [128, 512], BF16),
    ("mw1", [128, 128], BF16),
    ("mb1p", [128, 4], F32),
    ("Wmo", [128, 512], BF16),
    ("bmo", [128, 1], F32),
    ("cw1", [128, 128], BF16),
    ("cb1", [128, 1], F32),
    ("cw2g", [128, 128], BF16),
    ("cw2b", [128, 128], BF16),
    ("cb2g1", [128, 1], F32),
    ("cb2b", [128, 1], F32),
]


def _bcast_inner(ap, n):
    """[.., Q] AP -> [.., Q, n] with a stride-0 inner dim (free broadcast)."""
    newap = [list(p) for p in ap.ap] + [[0, n]]
    return bass.AP(ap.tensor, ap.offset, newap)


def build_program():
    nc = bacc.Bacc()

    x_d = nc.declare_dram_parameter("x", [NQ, 3], F32, isOutput=False)
    xh_d = nc.declare_dram_parameter("xh", [NQ, DH], F32, isOutput=False)
    ctbl_d = nc.declare_dram_parameter("ctbl", [L, 2 * D], BF16, isOutput=False)
    p2t_d = nc.declare_dram_parameter("p2t", [3, L], F32, isOutput=False)
    w_d = {}
    for name, shape, dt in WSPECS:
        w_d[name] = nc.declare_dram_parameter(name, shape, dt, isOutput=False)
    out_d = nc.declare_dram_parameter("out", [NQ, DH], F32, isOutput=True)

    with tile.TileContext(nc) as tc:
        _emit(nc, tc, x_d, xh_d, ctbl_d, p2t_d, w_d, out_d)
    nc.compile()
    _optimize_act_table_loads(nc)
    return nc


def _optimize_act_table_loads(nc):
    """Remap ln-only/exp-only ACT table-set loads to the combined
    natural_log_exp_and_others set, then drop consecutive reloads of an
    already-resident set. The stock placement picks the first set
    containing each function, which costs a ~1.5us table DMA at every
    Ln<->Exp alternation in the LN-rstd / softmax-exp blocks."""
    from concourse.hw_specs import get_activation_tables

    names = list(get_activation_tables(nc.m.arch).keys())
    try:
        ln_id = names.index("natural_log")
        exp_id = names.index("exp_and_others")
        combo_id = names.index("natural_log_exp_and_others")
    except ValueError:
        return
    for fn in nc.m.functions:
        for blk in fn.blocks:
            dead = []
            cur = {}
            for inst in blk.instructions:
                if isinstance(inst, mybir.InstLoadActFuncSet):
                    if inst.act_func_set_id in (ln_id, exp_id):
                        inst.act_func_set_id = combo_id
                    e = str(inst.engine)
                    si = inst.sync_info
                    clean = si is None or (
                        len(si.on_wait) == 0 and len(si.on_update) == 0
                    )
                    if cur.get(e) == inst.act_func_set_id and clean:
                        dead.append(inst)
                    else:
                        cur[e] = inst.act_func_set_id
            for inst in dead:
                blk.instructions.remove(inst)


def _emit(nc, tc, x_d, xh_d, ctbl_d, p2t_d, w_d, out_d):
    const = tc.alloc_tile_pool(name="const", bufs=1)
    wpool = tc.alloc_tile_pool(name="wpool", bufs=1)
    core = tc.alloc_tile_pool(name="core", bufs=1)
    tl = tc.alloc_tile_pool(name="tl", bufs=1)
    ck = tc.alloc_tile_pool(name="ck", bufs=1)
    psp = tc.alloc_tile_pool(name="psp", bufs=1, space="PSUM")
    _pools = [const, wpool, core, tl, ck, psp]

    _psn = [0]

    def PS(shape, tag, bufs, dtype=F32):
        _psn[0] += 1
        return psp.tile(
            shape, dtype, space="PSUM", tag=tag, bufs=bufs, name=f"ps_{tag}_{_psn[0]}"
        )

    # ---------- constants ----------
    ident = const.tile([128, 128], F32)
    make_identity(nc, ident[:])

    ones_col_bf = const.tile([128, 1], BF16)
    nc.vector.memset(ones_col_bf[:], 1.0)
    inv128_bf = const.tile([128, 1], BF16)
    nc.vector.memset(inv128_bf[:], 1.0 / 128.0)
    ones_row_bf = const.tile([1, 128], BF16)
    nc.vector.memset(ones_row_bf[:], 1.0)
    onesmat_bf = const.tile([128, 128], BF16)
    nc.vector.memset(onesmat_bf[:], 1.0)
    eps_col = const.tile([128, 1], F32)
    nc.vector.memset(eps_col[:], EPS)
    negpi_col = const.tile([128, 1], F32)
    nc.vector.memset(negpi_col[:], -np.pi)
    zeros_row_bf = const.tile([1, 128], BF16)
    nc.vector.memset(zeros_row_bf[:], 0.0)
    zeros_row512_bf = const.tile([1, 512], BF16)
    nc.vector.memset(zeros_row512_bf[:], 0.0)

    # ---------- weights ----------
    W = {}
    for name, shape, dt in WSPECS:
        wt = wpool.tile(shape, dt, name=f"w_{name}", tag=f"w_{name}")
        nc.sync.dma_start(out=wt[:], in_=w_d[name][:])
        W[name] = wt

    def Wh(name, h, w=128):
        return W[name][:, h * w : (h + 1) * w]

    # ---------- per-core precompute ----------
    # x arrives with a ones column appended (for the |p|^2 fold in scores)
    x_sb = core.tile([128, QT, 3], F32)
    nc.sync.dma_start(out=x_sb[:], in_=x_d[:].rearrange("(t q) c -> q t c", q=128))
    xsq = core.tile([128, QT], F32)
    xs2 = core.tile([128, QT, CD], F32)
    nc.vector.tensor_tensor(
        out=xs2[:], in0=x_sb[:, :, 0:CD], in1=x_sb[:, :, 0:CD], op=OP.mult
    )
    nc.vector.tensor_reduce(out=xsq[:], in_=xs2[:], axis=AX.X, op=OP.add)

    x_fm = core.tile([3, NQ], F32)
    for t in range(QT):
        tp = PS([3, 128], "mm", 4)
        nc.tensor.transpose(out=tp[:], in_=x_sb[:, t, :], identity=ident[:])
        nc.vector.tensor_copy(out=x_fm[:, 128 * t : 128 * (t + 1)], in_=tp[:])

    p2_fm = core.tile([3, L], F32)
    nc.sync.dma_start(out=p2_fm[:], in_=p2t_d[:])

    # ---------- cFFN on x_h (512 queries at once) ----------
    xh_fm = core.tile([128, NQ], BF16)
    xh_rm = core.tile([128, QT, DH], F32)
    nc.sync.dma_start(out=xh_rm[:], in_=xh_d[:].rearrange("(t q) c -> q t c", q=128))
    for t in range(QT):
        tp = PS([128, 128], "mm", 4)
        nc.tensor.transpose(out=tp[:], in_=xh_rm[:, t, :], identity=ident[:])
        nc.vector.tensor_copy(out=xh_fm[:, 128 * t : 128 * (t + 1)], in_=tp[:])

    c1ps = PS([128, NQ], "mm", 4)
    nc.tensor.matmul(out=c1ps[:], lhsT=W["cw1"][:], rhs=xh_fm[:], start=True, stop=True)
    c1 = core.tile([128, NQ], BF16)
    nc.scalar.activation(out=c1[:], in_=c1ps[:], func=GELU, bias=W["cb1"][:])

    cmean = PS([128, NQ], "rows4", 2)
    nc.tensor.matmul(out=cmean[0:1, :], lhsT=inv128_bf[:], rhs=c1[:], start=True, stop=True)
    c1sq = core.tile([128, NQ], BF16)
    nc.vector.tensor_tensor(out=c1sq[:], in0=c1[:], in1=c1[:], op=OP.mult)
    cmsq = PS([128, NQ], "rows4", 2)
    nc.tensor.matmul(out=cmsq[0:1, :], lhsT=inv128_bf[:], rhs=c1sq[:], start=True, stop=True)

    cm2 = core.tile([1, NQ], F32)
    nc.scalar.square(out=cm2[:], in_=cmean[0:1, :])
    cvar = core.tile([1, NQ], F32)
    nc.vector.tensor_tensor(out=cvar[:], in0=cmsq[0:1, :], in1=cm2[:], op=OP.subtract)
    # rstd = exp(-0.5*ln(var+eps)); stays in the ln/exp table set
    clnv = core.tile([1, NQ], F32)
    nc.scalar.activation(out=clnv[:], in_=cvar[:], func=AF.Ln, bias=eps_col[0:1, :])
    crstd_bf = core.tile([1, NQ], BF16)
    nc.scalar.activation(out=crstd_bf[:], in_=clnv[:], func=AF.Exp, scale=-0.5)
    cmr_bf = core.tile([1, NQ], BF16)
    nc.vector.tensor_tensor(out=cmr_bf[:], in0=cmean[0:1, :], in1=crstd_bf[:], op=OP.mult)
    crbc = PS([128, NQ], "mm", 4)
    nc.tensor.matmul(out=crbc[:], lhsT=ones_row_bf[:], rhs=crstd_bf[:], start=True, stop=True)
    cmbc = PS([128, NQ], "mm", 4)
    nc.tensor.matmul(out=cmbc[:], lhsT=ones_row_bf[:], rhs=cmr_bf[:], start=True, stop=True)
    z1c = core.tile([128, NQ], BF16)
    nc.vector.tensor_tensor(out=z1c[:], in0=c1[:], in1=crbc[:], op=OP.mult)
    zc = core.tile([128, NQ], BF16)
    nc.vector.tensor_tensor(out=zc[:], in0=z1c[:], in1=cmbc[:], op=OP.subtract)

    gp1 = core.tile([128, NQ], BF16)
    modadd = core.tile([128, NQ], BF16)
    gps_ = PS([128, NQ], "mm", 4)
    nc.tensor.matmul(out=gps_[:], lhsT=W["cw2g"][:], rhs=zc[:], start=True, stop=True)
    nc.scalar.activation(out=gp1[:], in_=gps_[:], func=AF.Identity, bias=W["cb2g1"][:])
    btps = PS([128, NQ], "mm", 4)
    nc.tensor.matmul(out=btps[:], lhsT=W["cw2b"][:], rhs=zc[:], start=True, stop=True)
    bt = core.tile([128, NQ], BF16)
    nc.scalar.activation(out=bt[:], in_=btps[:], func=AF.Identity, bias=W["cb2b"][:])
    ma1 = core.tile([128, NQ], BF16)
    nc.vector.tensor_scalar(
        out=ma1[:], in0=gp1[:], scalar1=W["ev_b2"][:], scalar2=None, op0=OP.mult
    )
    nc.vector.tensor_tensor(out=modadd[:], in0=ma1[:], in1=bt[:], op=OP.add)

    # deferred-stage state carried across chunks: (attps, mmean, msqp, gms,
    # qsl, zacc, den_t); tile epilogues are likewise deferred until after
    # the tile's chunk-3 deferred stage has been emitted.
    carry = [None]
    pending_epi = [None]

    def deferred_stage(state):
        """ln/exp block + weighted-sum DVE work for a completed chunk.

        Emitted one chunk later so mrstd/att_e share the ln/exp ACT table
        set with the next chunk's irstd."""
        attps, mmean, msqp, gms, qsl, zacc, den_t = state

        # mm2/mvar/mrstd: only rows {0,32,64,96} carry real stats; other
        # rows hold stale/garbage psum values whose results are never read.
        mm2 = ck.tile([128, CR], F32, tag="mm2")
        nc.scalar.square(out=mm2[:], in_=mmean[:])
        mvar = ck.tile([128, CR], F32, tag="mvar")
        nc.vector.tensor_tensor(out=mvar[:], in0=msqp[:], in1=mm2[:], op=OP.subtract)
        mlnv = ck.tile([128, CR], F32, tag="mlnv")
        nc.scalar.activation(out=mlnv[:], in_=mvar[:], func=AF.Ln, bias=eps_col[:])
        mrstd = ck.tile([128, CR], BF16, tag="mrstd")
        nc.scalar.activation(out=mrstd[:], in_=mlnv[:], func=AF.Exp, scale=-0.5)

        att_e = ck.tile([128, CR], BF16, tag="att_e")
        nc.scalar.activation(out=att_e[:], in_=attps[:], func=AF.Exp, bias=W["attconst"][:])
        nc.vector.tensor_reduce(
            out=den_t[:, qsl], in_=att_e[:].rearrange("p (a b) -> p a b", a=CQ),
            axis=AX.X, op=OP.add,
        )

        a2 = ck.tile([128, CR], BF16, tag="a2")
        nc.vector.tensor_tensor(out=a2[:], in0=att_e[:], in1=mrstd[:], op=OP.mult)
        # a3 = att_e * mean * rstd = a2 * mean  (psum operand)
        a3 = ck.tile([128, CR], BF16, tag="a3")
        nc.vector.tensor_tensor(out=a3[:], in0=a2[:], in1=mmean[:], op=OP.mult)
        s3bf = ck.tile([128, CQ], BF16, tag="s3bf")
        with nc.allow_low_precision(reason="16-term bf16 row sum; folded into bf16 matmul anyway"):
            nc.vector.tensor_reduce(
                out=s3bf[:], in_=a3[:].rearrange("p (a b) -> p a b", a=CQ), axis=AX.X,
                op=OP.add,
            )

        for h in range(H):
            a2bc = PS([128, CR], "mm", 4)
            nc.tensor.matmul(
                out=a2bc[:], lhsT=onesmat_bf[32 * h : 32 * h + 1, :],
                rhs=a2[32 * h : 32 * h + 1, :], start=True, stop=True,
                tile_position=(32 * h, 0),
            )
            zp = ck.tile([128, CR], BF16, tag="zp", bufs=2)
            nc.vector.tensor_tensor(out=zp[:], in0=gms[h][:], in1=a2bc[:], op=OP.mult)
            nc.vector.tensor_reduce(
                out=zacc[h][:, qsl], in_=zp[:].rearrange("p (a b) -> p a b", a=CQ),
                axis=AX.X, op=OP.add,
            )
            s3bc = PS([128, CQ], "mm", 4)
            nc.tensor.matmul(
                out=s3bc[:], lhsT=onesmat_bf[32 * h : 32 * h + 1, :],
                rhs=s3bf[32 * h : 32 * h + 1, :], start=True, stop=True,
                tile_position=(32 * h, 0),
            )
            nc.vector.tensor_tensor(
                out=zacc[h][:, qsl], in0=zacc[h][:, qsl], in1=s3bc[:], op=OP.subtract
            )

    def emit_epilogue(state):
        """Softmax denominator fold + output matmul/transpose/store for a
        finished tile (emitted after the tile's chunk-3 deferred stage)."""
        zacc, den_t, qs = state
        rden_t = tl.tile([128, 128], F32, tag="rden_t", bufs=2)
        nc.vector.reciprocal(out=rden_t[:], in_=den_t[:])
        rdbf = tl.tile([128, 128], BF16, tag="rdbf", bufs=2)
        nc.vector.tensor_copy(out=rdbf[:], in_=rden_t[:])
        for h in range(H):
            rdbc = PS([128, 128], "mm", 4)
            nc.tensor.matmul(
                out=rdbc[:], lhsT=onesmat_bf[32 * h : 32 * h + 1, :],
                rhs=rdbf[32 * h : 32 * h + 1, :], start=True, stop=True,
                tile_position=(32 * h, 0),
            )
            nc.vector.tensor_tensor(out=zacc[h][:], in0=zacc[h][:], in1=rdbc[:], op=OP.mult)

        outps = PS([128, 128], "mm", 4)
        for h in range(H):
            zbf = tl.tile([128, 128], BF16, tag="zbf")
            nc.vector.tensor_copy(out=zbf[:], in_=zacc[h][:])
            nc.tensor.matmul(
                out=outps[:], lhsT=Wh("Wmo", h), rhs=zbf[:], start=(h == 0), stop=(h == H - 1)
            )
        outsb = tl.tile([128, 128], F32, tag="outsb")
        nc.scalar.activation(out=outsb[:], in_=outps[:], func=AF.Identity, bias=W["bmo"][:])
        trp = PS([128, 128], "mm", 4)
        nc.tensor.transpose(out=trp[:], in_=outsb[:], identity=ident[:])
        outrm = tl.tile([128, 128], F32, tag="outrm")
        nc.vector.tensor_copy(out=outrm[:], in_=trp[:])
        nc.sync.dma_start(out=out_d[qs, :], in_=outrm[:])

    # ---------- prologue: scores -> top-16 -> gathers for ALL tiles ----
    # Emitting every tile's gather chain up front queues all 16 gathers on
    # the gpsimd engine from the start, so no tile boundary ever stalls
    # waiting for its gather (the chunk pipeline consumes gather #k around
    # t~60k us while it completes around t~11k us).
    cfm_all = []
    gw_all = []
    for t in range(QT):
        qs = slice(128 * t, 128 * (t + 1))

        # scores (two 512-wide halves; |p|^2 folded via ones row) + top-16
        scores = tl.tile([128, L], F32, tag="scores")
        for s in range(2):
            sl = slice(512 * s, 512 * (s + 1))
            scps = PS([128, 512], "mm", 4)
            nc.tensor.matmul(out=scps[:], lhsT=x_fm[:, qs], rhs=p2_fm[:, sl], start=True, stop=True)
            nc.scalar.copy(out=scores[:, sl], in_=scps[:])
        vals = tl.tile([128, K], F32, tag="vals", bufs=2)
        idxs = tl.tile([128, K], U32, tag="idxs", bufs=2)
        scr2 = tl.tile([128, L], F32, tag="scr2")
        nc.vector.max(out=vals[:, 0:8], in_=scores[:])
        nc.vector.max_index(out=idxs[:, 0:8], in_max=vals[:, 0:8], in_values=scores[:])
        nc.vector.match_replace(
            out=scr2[:], in_to_replace=vals[:, 0:8], in_values=scores[:], imm_value=-1e30
        )
        nc.vector.max(out=vals[:, 8:16], in_=scr2[:])
        nc.vector.max_index(out=idxs[:, 8:16], in_max=vals[:, 8:16], in_values=scr2[:])

        # index prep for dma_gather: idx16[k, q] (int16) replicated across
        # the 8 gpsimd cores' 16-partition blocks
        idxf = tl.tile([128, K], F32, tag="idxf", bufs=2)
        nc.vector.tensor_copy(out=idxf[:], in_=idxs[:])
        idxt_ps = PS([K, 128], "mm", 4)
        nc.tensor.transpose(out=idxt_ps[:], in_=idxf[:], identity=ident[:])
        idx16 = tl.tile([128, 128], I16, tag="idx16", bufs=QT)
        nc.vector.tensor_copy(out=idx16[0:16, :], in_=idxt_ps[:])
        for b in range(1, 8):
            nc.sync.dma_start(out=idx16[16 * b : 16 * b + 16, :], in_=idx16[0:16, :])

        # transposed gathers (512B rows): cfm [128, 2, 512] per chunk:
        # block 0 = c features; block 1 rows 0-3 = w1v.c per head, rows
        # 32-37 = p hi/lo splits, row 64 = 0.5/sigma^2 (row choices keep
        # every consumer's partition bases legal/aligned)
        cfm_cs = []
        for gc in range(NCH):
            cfm_c = tl.tile([128, 2, CR], BF16, tag=f"cfm{gc}", bufs=3,
                            name=f"cfm{gc}_{t}")
            nc.gpsimd.dma_gather(
                out_ap=cfm_c[:], in_ap=ctbl_d[:],
                idxs_ap=idx16[:, 32 * gc : 32 * gc + 32], num_idxs=CR,
                num_idxs_reg=CR, elem_size=2 * D, transpose=True,
            )
            cfm_cs.append(cfm_c)
        cfm_all.append(cfm_cs)

        # -d^2 (q-major, [128, K]); flattened to a row in the main loop
        negd2 = tl.tile([128, K], BF16, tag="negd2", bufs=QT)
        nc.vector.tensor_scalar(
            out=negd2[:], in0=vals[:], scalar1=xsq[:, t : t + 1], scalar2=None,
            op0=OP.subtract,
        )
        gw_all.append(negd2)

    # ---------- per query tile ----------
    for t in range(QT):
        qs = slice(128 * t, 128 * (t + 1))
        cfm_cs = cfm_all[t]
        negd2 = gw_all[t]

        # -d^2 row DMA-flattened INTO partition 64 so the gw multiply is
        # partition-base-aligned with the gathered 0.5/sigma^2 row.
        # Emitted here, not in the prologue, so the in-order DVE queue
        # never blocks on a still-running gather.
        negd2_row = tl.tile([65, TR], BF16, tag="negd2_row", bufs=2)
        nc.sync.dma_start(out=negd2_row[64:65, :], in_=negd2[:])
        gw_t = tl.tile([65, TR], BF16, tag="gw_t", bufs=2)
        for gc in range(NCH):
            nc.vector.tensor_tensor(
                out=gw_t[64:65, CR * gc : CR * (gc + 1)],
                in0=negd2_row[64:65, CR * gc : CR * (gc + 1)],
                in1=cfm_cs[gc][64:65, 1, :], op=OP.mult,
            )

        # t_x = x @ Bs for this tile [128, 128]
        txps = PS([128, 128], "mm", 4)
        nc.tensor.matmul(out=txps[:], lhsT=W["rff"][:], rhs=x_fm[0:2, qs], start=True, stop=True)
        t_x = tl.tile([128, 128], F32, tag="t_x", bufs=2)
        nc.vector.tensor_copy(out=t_x[:], in_=txps[:])

        # ---- hoisted RFF features for all 4 chunks; the 16 Sin calls land
        # contiguously in the scalar queue (one trig table load per tile)
        fqs, fvs = [], []
        for c in range(NCH):
            q0 = CQ * c
            qsl = slice(q0, q0 + CQ)
            cs = slice(CR * c, CR * (c + 1))
            # p@Bs from hi/lo split: 6-row bf16 matmul (table rows 32-37)
            pbs = PS([128, CR], "mm", 4)
            nc.tensor.matmul(
                out=pbs[:], lhsT=W["rff6"][32:38, :], rhs=cfm_cs[c][32:38, 1, :],
                start=True, stop=True,
            )
            # frac chain; paired ops share a tag (lifetimes don't overlap)
            tfull = ck.tile([128, CQ, K], F32, tag="tfrac", bufs=2)
            nc.vector.tensor_tensor(
                out=tfull[:], in0=_bcast_inner(t_x[:, qsl], K),
                in1=pbs[:].rearrange("p (a b) -> p a b", a=CQ), op=OP.subtract,
            )
            ti = ck.tile([128, CQ, K], I32, tag="ifrac", bufs=2)
            nc.vector.tensor_copy(out=ti[:], in_=tfull[:])
            fs = ck.tile([128, CQ, K], F32, tag="sfrac", bufs=2)
            nc.vector.tensor_tensor(out=fs[:], in0=tfull[:], in1=ti[:], op=OP.subtract)
            dc0 = ck.tile([128, CQ, K], F32, tag="tfrac", bufs=2)
            nc.vector.tensor_scalar(
                out=dc0[:], in0=tfull[:], scalar1=0.25, scalar2=None, op0=OP.add
            )
            ui = ck.tile([128, CQ, K], I32, tag="ifrac", bufs=2)
            nc.vector.tensor_copy(out=ui[:], in_=dc0[:])
            dc = ck.tile([128, CQ, K], F32, tag="sfrac", bufs=2)
            nc.vector.tensor_tensor(out=dc[:], in0=dc0[:], in1=ui[:], op=OP.subtract)
            fq = tl.tile([128, CR], BF16, tag=f"fq{c}", bufs=2, name=f"fq{c}_{t}")
            fv = tl.tile([128, CR], BF16, tag=f"fv{c}", bufs=2, name=f"fv{c}_{t}")
            fs2 = fs[:].rearrange("p a b -> p (a b)")
            dc2 = dc[:].rearrange("p a b -> p (a b)")
            nc.scalar.activation(out=fq[0:64, :], in_=fs2[0:64, :], func=AF.Sin, scale=TWO_PI)
            nc.scalar.activation(out=fq[64:128, :], in_=dc2[0:64, :], func=AF.Sin, scale=TWO_PI)
            nc.scalar.activation(out=fv[0:64, :], in_=fs2[64:128, :], func=AF.Sin, scale=TWO_PI)
            nc.scalar.activation(out=fv[64:128, :], in_=dc2[64:128, :], func=AF.Sin, scale=TWO_PI)
            fqs.append(fq)
            fvs.append(fv)

        zacc = [
            tl.tile([128, 128], F32, tag=f"zacc{h}", name=f"zacc{h}_{t}", bufs=2)
            for h in range(H)
        ]
        den_t = tl.tile([128, 128], F32, tag="den_t", bufs=2, name=f"den_{t}")

        # ---------- chunks ----------
        for c in range(NCH):
            q0 = CQ * c
            qsl = slice(q0, q0 + CQ)
            gq = slice(128 * t + q0, 128 * t + q0 + CQ)
            cs = slice(CR * c, CR * (c + 1))
            cg2 = cfm_cs[c][:, 0, :]

            # q path
            g2ps = PS([128, CR], "mm", 4)
            nc.tensor.matmul(out=g2ps[:], lhsT=W["eq_w1"][:], rhs=fqs[c][:], start=True, stop=True)
            g2 = ck.tile([128, CR], BF16, tag="g2", bufs=2)
            nc.scalar.activation(out=g2[:], in_=g2ps[:], func=GELU, bias=W["eq_b1"][:])

            # attention logits in one psum tile, rows {0,32,64,96}.
            # seed = 0.5*gw broadcast to all rows (start=True).
            attps = PS([128, CR], "attps", 2)
            nc.tensor.matmul(
                out=attps[:], lhsT=onesmat_bf[64:65, :],
                rhs=gw_t[64:65, cs], start=True, stop=False,
                skip_group_check=True,
            )
            # w2v^T g2 for all heads [4, CR] + gathered w1v.c, scattered to
            # rows {32h} by one selector matmul
            w24 = PS([128, CR], "mm", 4)
            nc.tensor.matmul(out=w24[0:4, :], lhsT=W["w2v"][:], rhs=g2[:], start=True, stop=True)
            wsum = ck.tile([4, CR], BF16, tag="wsum", bufs=2)
            nc.vector.tensor_tensor(
                out=wsum[:], in0=w24[0:4, :], in1=cfm_cs[c][0:4, 1, :], op=OP.add
            )
            nc.tensor.matmul(
                out=attps[:], lhsT=W["sel4"][:], rhs=wsum[:],
                start=False, stop=False, skip_group_check=True,
            )
            for h in range(H):
                ups = PS([128, CR], "mm", 4)
                nc.tensor.matmul(out=ups[:], lhsT=Wh("Mq", h), rhs=g2[:], start=True, stop=True)
                qkp = ck.tile([128, CR], BF16, tag="qkp", bufs=2)
                nc.vector.tensor_tensor(out=qkp[:], in0=ups[:], in1=cg2, op=OP.mult)
                nc.tensor.matmul(
                    out=attps[32 * h : 32 * h + 1, :], lhsT=ones_col_bf[:], rhs=qkp[:],
                    start=False, stop=(h == H - 1), tile_position=(0, 32 * h),
                    skip_group_check=True,
                )

            # v path
            ev1ps = PS([128, CR], "mm", 4)
            nc.tensor.matmul(out=ev1ps[:], lhsT=W["ev_w1"][:], rhs=fvs[c][:], start=True, stop=True)
            ev1 = ck.tile([128, CR], BF16, tag="ev1", bufs=2)
            nc.scalar.activation(out=ev1[:], in_=ev1ps[:], func=GELU, bias=W["ev_b1"][:])
            ev2ps = PS([128, CR], "mm", 4)
            nc.tensor.matmul(out=ev2ps[:], lhsT=W["ev_w2"][:], rhs=ev1[:], start=True, stop=True)
            mv = ck.tile([128, CQ, K], BF16, tag="mv", bufs=2)
            nc.vector.tensor_tensor(
                out=mv[:], in0=ev2ps[:].rearrange("p (a b) -> p a b", a=CQ),
                in1=_bcast_inner(gp1[:, gq], K), op=OP.mult,
            )
            ivin = ck.tile([128, CQ, K], BF16, tag="ivin", bufs=2)
            nc.vector.tensor_tensor(
                out=ivin[:], in0=mv[:], in1=_bcast_inner(modadd[:, gq], K), op=OP.add
            )
            iv1ps = PS([128, CR], "mm", 4)
            nc.tensor.matmul(
                out=iv1ps[:], lhsT=W["ivw1"][:],
                rhs=ivin[:].rearrange("p a b -> p (a b)"), start=True, stop=True,
            )
            iv1 = ck.tile([128, CR], BF16, tag="iv1", bufs=2)
            nc.scalar.activation(out=iv1[:], in_=iv1ps[:], func=GELU, bias=W["ivb1"][:])

            # iv LN (rstd via ln/exp; the ln/exp block below also carries the
            # previous chunk's deferred mrstd/att_e work). Mean at psum row
            # 0, mean-of-squares at row 32 of ONE tile (fewer psum allocs);
            # the var math runs at partition base 32 (legal base), with ACT
            # crossing partitions where needed.
            ivstats = PS([128, CR], "rows4", 2)
            nc.tensor.matmul(
                out=ivstats[0:1, :], lhsT=inv128_bf[:], rhs=iv1[:],
                start=True, stop=True, skip_group_check=True,
            )
            ivsq = ck.tile([128, CR], BF16, tag="ivsq", bufs=2)
            nc.vector.tensor_tensor(out=ivsq[:], in0=iv1[:], in1=iv1[:], op=OP.mult)
            nc.tensor.matmul(
                out=ivstats[32:33, :], lhsT=inv128_bf[:], rhs=ivsq[:],
                start=True, stop=True, tile_position=(0, 32),
                skip_group_check=True,
            )
            im2 = ck.tile([33, CR], F32, tag="im2")
            nc.scalar.square(out=im2[32:33, :], in_=ivstats[0:1, :])
            ivar = ck.tile([33, CR], F32, tag="ivar")
            nc.vector.tensor_tensor(
                out=ivar[32:33, :], in0=ivstats[32:33, :], in1=im2[32:33, :],
                op=OP.subtract,
            )

            ilnv = ck.tile([1, CR], F32, tag="ilnv")
            nc.scalar.activation(out=ilnv[:], in_=ivar[32:33, :], func=AF.Ln, bias=eps_col[0:1, :])
            irstd_bf = ck.tile([1, CR], BF16, tag="irstd_bf")
            nc.scalar.activation(out=irstd_bf[:], in_=ilnv[:], func=AF.Exp, scale=-0.5)

            # deferred ln/exp + weighted-sum work of the previous chunk,
            # then (entering a new tile) the previous tile's epilogue
            if carry[0] is not None:
                deferred_stage(carry[0])
                carry[0] = None
            if c == 0 and pending_epi[0] is not None:
                emit_epilogue(pending_epi[0])
                pending_epi[0] = None

            imr_bf = ck.tile([1, CR], BF16, tag="imr_bf")
            nc.vector.tensor_tensor(
                out=imr_bf[:], in0=ivstats[0:1, :], in1=irstd_bf[:], op=OP.mult
            )
            irbc = PS([128, CR], "mm", 4)
            nc.tensor.matmul(out=irbc[:], lhsT=ones_row_bf[:], rhs=irstd_bf[:], start=True, stop=True)
            imbc = PS([128, CR], "mm", 4)
            nc.tensor.matmul(out=imbc[:], lhsT=ones_row_bf[:], rhs=imr_bf[:], start=True, stop=True)
            z1 = ck.tile([128, CR], BF16, tag="z1", bufs=2)
            nc.vector.tensor_tensor(out=z1[:], in0=iv1[:], in1=irbc[:], op=OP.mult)
            ziv = ck.tile([128, CR], BF16, tag="ziv", bufs=2)
            nc.vector.tensor_tensor(out=ziv[:], in0=z1[:], in1=imbc[:], op=OP.subtract)

            # per-head v1 -> gm ; m-LN stats to psum rows {32h}.
            # DVE memset seeds the rows the per-head matmuls don't write
            # (their values are never used, but reads must be initialized).
            # The WA (conditioning-gate constant) term is folded into the
            # vg->bf16 copy as a per-feature bias: m1 = mw1^T((g^+c1).v0)
            # + WB^T ziv, saving one PE stream per head.
            gms = []
            mmean = PS([128, CR], "rows4", 2)
            msqp = PS([128, CR], "rows4", 2)
            nc.vector.memset(mmean[:], 0.0)
            nc.vector.memset(msqp[:], 0.0)
            for h in range(H):
                vgps = PS([128, CR], "mm", 4)
                nc.tensor.matmul(out=vgps[:], lhsT=Wh("ivw2g", h), rhs=ziv[:], start=True, stop=True)
                gc_sb = ck.tile([128, CR], BF16, tag="gc_sb", bufs=2)
                nc.scalar.activation(
                    out=gc_sb[:], in_=vgps[:], func=AF.Identity,
                    bias=W["c1col"][:, h : h + 1],
                )
                v0ps = PS([128, CR], "mm", 4)
                nc.tensor.matmul(out=v0ps[:], lhsT=Wh("wv", h), rhs=cg2, start=True, stop=True)
                p_sb = ck.tile([128, CR], BF16, tag="p_sb", bufs=2)
                nc.vector.tensor_tensor(out=p_sb[:], in0=v0ps[:], in1=gc_sb[:], op=OP.mult)
                m1ps = PS([128, CR], "mm", 4)
                nc.tensor.matmul(out=m1ps[:], lhsT=W["mw1"][:], rhs=p_sb[:], start=True, stop=False)
                nc.tensor.matmul(out=m1ps[:], lhsT=Wh("WB", h), rhs=ziv[:], start=False, stop=True)
                gm = ck.tile([128, CR], BF16, tag=f"gm{h}", bufs=2)
                nc.scalar.activation(out=gm[:], in_=m1ps[:], func=GELU, bias=W["mb1p"][:, h : h + 1])
                gms.append(gm)
                nc.tensor.matmul(
                    out=mmean[32 * h : 32 * h + 1, :], lhsT=inv128_bf[:], rhs=gm[:],
                    start=True, stop=True, tile_position=(0, 32 * h),
                    skip_group_check=True,
                )
                gsq = ck.tile([128, CR], BF16, tag="gsq", bufs=2)
                nc.vector.tensor_tensor(out=gsq[:], in0=gm[:], in1=gm[:], op=OP.mult)
                nc.tensor.matmul(
                    out=msqp[32 * h : 32 * h + 1, :], lhsT=inv128_bf[:], rhs=gsq[:],
                    start=True, stop=True, tile_position=(0, 32 * h),
                    skip_group_check=True,
                )

            # spill the stats to SBUF right away: frees the psum bank a
            # chunk earlier (deferred stage reads the SBUF copies), which
            # unblocks the next chunk's iv-stat matmuls
            mmean_sb = ck.tile([128, CR], F32, tag="mmean_sb", bufs=2)
            nc.scalar.copy(out=mmean_sb[:], in_=mmean[:])
            msqp_sb = ck.tile([128, CR], F32, tag="msqp_sb", bufs=2)
            nc.scalar.copy(out=msqp_sb[:], in_=msqp[:])

            carry[0] = (attps, mmean_sb, msqp_sb, gms, qsl, zacc, den_t)

        # epilogue for THIS tile can only be emitted after its chunk-3
        # deferred stage (inside the next tile's first chunk, or right
        # here for the last tile)
        pending_epi[0] = (zacc, den_t, qs)

    deferred_stage(carry[0])
    carry[0] = None
    emit_epilogue(pending_epi[0])
    pending_epi[0] = None

    for p in reversed(_pools):
        p.release()


# ======================= host side =======================


def _host_prep(inputs):
    f = {k: np.asarray(v, np.float32) for k, v in inputs.items()}

    def bf(x):
        return np.ascontiguousarray(np.asarray(x, np.float32)).astype(ml_dtypes.bfloat16)

    def col(x):
        return np.ascontiguousarray(np.asarray(x, np.float32).reshape(-1, 1))

    rff = np.concatenate([FQ * f["rffq"], FV * f["rffv"]], axis=1)  # [2,128]
    # hi/lo split of rff rows for the on-device p@Bs matmul:
    # (r_hi + r_lo)(p_hi + p_lo) ~ r_hi*p_hi + r_hi*p_lo + r_lo*p_hi
    rhi = rff.astype(ml_dtypes.bfloat16).astype(np.float32)
    rlo = rff - rhi
    rff6 = np.zeros((38, 128), np.float32)
    rff6[32] = rhi[0]
    rff6[33] = rhi[0]
    rff6[34] = rlo[0]
    rff6[35] = rhi[1]
    rff6[36] = rhi[1]
    rff6[37] = rlo[1]

    sel4 = np.zeros((4, 128), np.float32)
    for h in range(H):
        sel4[h, 32 * h] = 1.0

    wq_s = f["wq"] * SCALE
    bq_s = f["bq"] * SCALE
    W_qm = f["eq_w2"] @ wq_s
    b_qm = f["eq_b2"] @ wq_s + bq_s
    Mq = np.zeros((128, 512), np.float32)
    w1v = np.zeros((128, 4), np.float32)  # per-head w1v vectors (feature dim)
    w2v = np.zeros((128, 4), np.float32)
    attconst = np.zeros((128, 1), np.float32)
    for h in range(H):
        sl = slice(128 * h, 128 * (h + 1))
        Wq_h = W_qm[:, sl]
        wk_h = f["wk"][:, sl]
        bk_h = f["bk"][sl]
        bq_h = b_qm[sl]
        Mq[:, sl] = Wq_h @ wk_h.T
        w1v[:, h] = wk_h @ bq_h
        w2v[:, h] = Wq_h @ bk_h
        attconst[32 * h, 0] = float(bq_h @ bk_h)

    ivw2f = f["ivls"][:, None] * f["ivw2"]
    ivb2f = f["ivb2"] + f["ivlb"] @ f["ivw2"]
    ivw2g = ivw2f[:, :HD]
    ivw2b = ivw2f[:, HD:]
    # bilinear expansion: m1 = mw1.T ((vg+c1)*v0) + WB.T ziv + mb1p
    # (the WA = wv diag(c1) mw1 term folds into the vg copy's c1 bias)
    c1col = np.zeros((128, H), np.float32)
    WB = np.zeros((128, 512), np.float32)
    mb1p = np.zeros((128, H), np.float32)
    for h in range(H):
        sl = slice(128 * h, 128 * (h + 1))
        c1_h = 1.0 + ivb2f[:HD][sl]
        bv_h = f["bv"][sl]
        b2_h = ivb2f[HD:][sl]
        c1col[:, h] = c1_h
        WB[:, sl] = (ivw2g[:, sl] @ np.diag(bv_h) + ivw2b[:, sl]) @ f["mw1"]
        mb1p[:, h] = f["mb1"] + (bv_h * c1_h + b2_h) @ f["mw1"]

    mw2f = f["mls"][:, None] * f["mw2"]
    mb2f = f["mb2"] + f["mlb"] @ f["mw2"]
    Wmo = np.zeros((128, 512), np.float32)
    for h in range(H):
        wo_h = f["wo"][128 * h : 128 * (h + 1), :]
        Wmo[:, 128 * h : 128 * (h + 1)] = mw2f @ wo_h
    bmo = f["bo"] + sum(mb2f @ f["wo"][128 * h : 128 * (h + 1), :] for h in range(H))

    cw2f = f["cls"][:, None] * f["cw2"]
    cb2f = f["cb2"] + f["clb"] @ f["cw2"]

    weights = {
        "rff": np.ascontiguousarray(rff),
        "rff6": bf(rff6),
        "sel4": bf(sel4),
        "eq_w1": bf(f["eq_w1"]),
        "eq_b1": col(f["eq_b1"]),
        "Mq": bf(Mq),
        "w2v": bf(w2v),
        "attconst": attconst.astype(np.float32),
        "ev_w1": bf(f["ev_w1"]),
        "ev_b1": col(f["ev_b1"]),
        "ev_w2": bf(f["ev_w2"]),
        "ev_b2": col(f["ev_b2"]),
        "ivw1": bf(f["ivw1"]),
        "ivb1": col(f["ivb1"]),
        "ivw2g": bf(ivw2g),
        "wv": bf(f["wv"]),
        "c1col": np.ascontiguousarray(c1col),
        "WB": bf(WB),
        "mw1": bf(f["mw1"]),
        "mb1p": np.ascontiguousarray(mb1p),
        "Wmo": bf(Wmo),
        "bmo": col(bmo),
        "cw1": bf(f["cw1"]),
        "cb1": col(f["cb1"]),
        "cw2g": bf(cw2f[:, :DH]),
        "cw2b": bf(cw2f[:, DH:]),
        "cb2g1": col(cb2f[:DH] + 1.0),
        "cb2b": col(cb2f[DH:]),
    }

    x_flat = f["x"].reshape(B * N, CD)
    xh_flat = f["x_h"].reshape(B * N, DH)

    in_maps = []
    for i in range(NCORES):
        b = (i * NQ) // N
        rs = slice(i * NQ, (i + 1) * NQ)
        p_b = f["p"][b]
        c_b = f["c"][b]
        sig_b = f["window_sigma"][b]
        inv2 = 1.0 / (sig_b[:, 0] ** 2)
        phi = p_b.astype(ml_dtypes.bfloat16)
        plo = (p_b - phi.astype(np.float32)).astype(ml_dtypes.bfloat16)
        w1vc = (c_b @ w1v).astype(np.float32)  # [L, 4]
        ctbl = np.zeros((L, 2 * D), ml_dtypes.bfloat16)
        ctbl[:, :D] = bf(c_b)
        ctbl[:, D + 0 : D + 4] = bf(w1vc)
        ctbl[:, D + 32] = phi[:, 0]
        ctbl[:, D + 33] = plo[:, 0]
        ctbl[:, D + 34] = phi[:, 0]
        ctbl[:, D + 35] = phi[:, 1]
        ctbl[:, D + 36] = plo[:, 1]
        ctbl[:, D + 37] = phi[:, 1]
        ctbl[:, D + 64] = (0.5 * inv2).astype(ml_dtypes.bfloat16)
        p2t = np.zeros((3, L), np.float32)
        p2t[0:2] = (2.0 * p_b).T
        p2t[2] = -(p_b**2).sum(1)
        x3 = np.concatenate(
            [x_flat[rs], np.ones((NQ, 1), np.float32)], axis=1
        )
        m = {
            "x": np.ascontiguousarray(x3),
            "xh": np.ascontiguousarray(xh_flat[rs]),
            "ctbl": ctbl,
            "p2t": np.ascontiguousarray(p2t),
        }
        m.update(weights)
        in_maps.append(m)
    return in_maps


_PROGRAM_CACHE = {}


def kernel(**inputs):
    in_maps = _host_prep(inputs)
    if "nc" not in _PROGRAM_CACHE:
        _PROGRAM_CACHE["nc"] = build_program()
    nc = _PROGRAM_CACHE["nc"]

    from concourse.bass_utils import run_bass_kernel_spmd

    res = run_bass_kernel_spmd(nc, in_maps, core_ids=list(range(NCORES)))
    outs = [np.asarray(res.results[i]["out"], np.float32) for i in range(NCORES)]
    return np.concatenate(outs, axis=0).reshape(B, N, DH)
